# revision 1
# baseline (speedup 1.0000x reference)
"""Trainium2 Bass kernel for nn_CLIP_69458211111620.

Data-parallel over batch B=128 across 8 NeuronCores (16 batches/core).
All matmuls run in float32r (fp32 with 11-bit mantissa, full PE rate).
Weights are replicated; inputs/weights are pre-rounded to f32r on host so
DMA-loaded tiles can feed the PE directly.

Per-core pipeline:
  pass 1: sensorT = LN(x_enc[b].T @ W_emb).T   (staged to DRAM scratch, f32r)
  pass 2: QKV, cross-attention-with-DTB, LN, attention pooling -> pooledT
  pass 3: fc head (streamed Wf1/Wf2) -> rul[16,1]
"""
import sys

sys.path.insert(0, "/opt/trn_rl_repo")

import numpy as np

NCORES = 8
NB = 16          # batches per core
T, C, D, DF, H = 1024, 512, 1024, 2048, 64
ISD = 1.0 / 32.0  # 1/sqrt(D)
EPS = 1e-5


def _round_f32r(x):
    """Round fp32 array to f32r (11-bit mantissa, RNE), bit-exact with PE."""
    u = np.ascontiguousarray(x, dtype=np.float32).view(np.uint32).copy()
    lsb = (u >> np.uint32(12)) & np.uint32(1)
    u += np.uint32(0x7FF) + lsb
    u &= np.uint32(0xFFFFF000)
    return u.view(np.float32)


def _chunk_major(v, nchunk):
    """[nchunk*128] -> [128, nchunk] with t[p,k] = v[k*128+p]."""
    return np.ascontiguousarray(
        np.asarray(v, dtype=np.float32).reshape(nchunk, 128).T
    )


def _sbuf_layout(w, nk):
    """[nk*128, F] -> [128, nk*F]: partition-major SBUF image of the
    [128, nk, F] chunked tile (one flat contiguous DMA per load)."""
    w = np.asarray(w, dtype=np.float32)
    f = w.shape[1]
    return np.ascontiguousarray(
        w.reshape(nk, 128, f).transpose(1, 0, 2).reshape(128, nk * f)
    )


def _build(age_scale_f, bf3_f, bemb_nz, bv_nz, fold_qk=True, sim_acts=False):
    import concourse.tile as tile
    import concourse.bass as bass
    from concourse import bacc, mybir

    F32 = mybir.dt.float32
    F32R = mybir.dt.float32r
    BF16 = mybir.dt.bfloat16
    AF = mybir.ActivationFunctionType
    ALU = mybir.AluOpType
    AX = mybir.AxisListType
    ts = bass.ts
    # CoreSim lacks Gelu/Lrelu tables; substitute sim-supported funcs for
    # logic-only checking (HW semantics of Gelu/Lrelu validated separately).
    AF_GELU = AF.Tanh if sim_acts else AF.Gelu
    AF_LRELU = AF.Relu if sim_acts else AF.Lrelu

    I32 = mybir.dt.int32
    RSQRT_MAGIC = 0x5F3759DF

    nc = bacc.Bacc("TRN2", target_bir_lowering=False, debug=False)

    def inp(name, shape, dt=F32R):
        return nc.dram_tensor(name, shape, dt, kind="ExternalInput").ap()

    X = inp("x", (NB, 128, 8 * C))
    WEMB = inp("wemb", (128, 8 * D))
    WV = inp("wv", (128, 8 * D))
    if fold_qk:
        MQK = inp("m_mat", (128, 8 * D))   # Wq @ Wk^T (pre-laid-out)
        WQB = inp("wqb", (128, 8 * C))     # Wq @ basis^T (pre-laid-out)
        WQ = WK = BAST = None
    else:
        WQ, WK = inp("wq", (128, 8 * D)), inp("wk", (128, 8 * D))
        BAST = inp("bast", (128, 8 * C))
        MQK = WQB = None
    WP1G = inp("wp1g", (128, 8 * 128))  # H=64 pad 128; pre-laid-out
    WP2 = inp("wp2", (H, 1), BF16)
    WF1 = inp("wf1", (32, 128, 512))
    WF2 = inp("wf2", (64, 128, 512))
    WF3C = inp("wf3c", (128, DF // 128, 2))  # duplicated cols (even-N rule)
    GS = inp("gs_c", (128, 8), F32)
    BS = inp("bs_c", (128, 8), F32)
    GF = inp("gf_c", (128, 8), F32)
    BF = inp("bf_c", (128, 8), F32)
    if not fold_qk:
        BQ = inp("bq_c", (128, 8), F32)
        BK = inp("bk_c", (128, 8), F32)
    BP1E = inp("bp1e", (H, 1), F32)
    BF1 = inp("bf1_c", (128, 16), F32)
    BF2 = inp("bf2_c", (128, 16), F32)
    IDENT = inp("ident", (128, 128), F32)
    BEMB = inp("bemb_row", (1, D)) if bemb_nz else None
    BVR = inp("bv_row", (1, D)) if bv_nz else None
    RUL = nc.dram_tensor("rul", (NB, 1), F32, kind="ExternalOutput").ap()

    with tile.TileContext(nc) as tc:
        # ---- long-lived small tiles -----------------------------------
        glob = tc.alloc_tile_pool(name="glob", bufs=1)
        id_sb = glob.tile([128, 128], F32, name="id_sb")
        magic_t = glob.tile([128, 1], I32, name="magic_t")
        ages_t = glob.tile([128, 1], F32, name="ages_t")
        pooledT = glob.tile([128, 8, NB], F32R, name="pooledT")
        gf_sb = glob.tile([128, 8], F32, name="gf_sb")
        bf_sb = glob.tile([128, 8], F32, name="bf_sb")
        ones_r = glob.tile([1, 128], F32R, name="ones_r") if (bemb_nz or bv_nz) else None
        nc.sync.dma_start(id_sb[:], IDENT[:])
        nc.sync.dma_start(gf_sb[:], GF[:])
        nc.sync.dma_start(bf_sb[:], BF[:])
        nc.gpsimd.memset(magic_t[:], RSQRT_MAGIC)
        nc.gpsimd.memset(ages_t[:], age_scale_f)

        def emit_rsqrt(pool, v_ap, tagp):
            """out = 1/sqrt(v + EPS), fully on DVE (Quake seed + 2 Newton).
            Avoids ACT Ln/Exp table loads which thrash against softmax Exp."""
            ve = pool.tile([128, 1], F32, name=f"{tagp}ve", tag=f"{tagp}ve")
            nc.vector.tensor_scalar(ve[:], v_ap, EPS, None, op0=ALU.add)
            y = pool.tile([128, 1], F32, name=f"{tagp}y0", tag=f"{tagp}y0")
            nc.vector.tensor_scalar(
                y.bitcast(I32)[:], ve.bitcast(I32)[:], 1, None,
                op0=ALU.logical_shift_right,
            )
            nc.vector.scalar_tensor_tensor(
                y.bitcast(I32)[:], y.bitcast(I32)[:], -1, magic_t[:],
                op0=ALU.mult, op1=ALU.add,
            )
            for it in range(2):
                a = pool.tile([128, 1], F32, name=f"{tagp}a{it}", tag=f"{tagp}a{it}")
                nc.vector.tensor_tensor(a[:], y[:], y[:], op=ALU.mult)
                nc.vector.tensor_tensor(a[:], a[:], ve[:], op=ALU.mult)
                nc.vector.tensor_scalar(
                    a[:], a[:], -0.5, 1.5, op0=ALU.mult, op1=ALU.add
                )
                nc.vector.tensor_tensor(y[:], y[:], a[:], op=ALU.mult)
            return y
        if ones_r is not None:
            nc.gpsimd.memset(ones_r[:], 1.0)
        bemb_sb = None
        if bemb_nz:
            bemb_sb = glob.tile([1, D], F32R, name="bemb_sb")
            nc.sync.dma_start(bemb_sb[:], BEMB[:])
        bv_sb = None
        if bv_nz:
            bv_sb = glob.tile([1, D], F32R, name="bv_sb")
            nc.sync.dma_start(bv_sb[:], BVR[:])

        # ---- DRAM staging for sensorT ---------------------------------
        stg_pool = tc.alloc_tile_pool(name="stg", bufs=1, space="DRAM")
        stg = [
            stg_pool.tile([128, 8, C], F32R, name=f"stg{b}") for b in range(NB)
        ]

        # ---- pass-2 weights: allocate now, DMA trickled during pass 1 --
        p2w = tc.alloc_tile_pool(name="p2w", bufs=1)
        wv_sb = p2w.tile([128, 8, D], F32R, name="wv_sb")
        wp1_sb = p2w.tile([128, 8, 128], F32R, name="wp1_sb")
        wp2_sb = p2w.tile([H, 1], BF16, name="wp2_sb")
        bp1_sb = p2w.tile([H, 1], F32, name="bp1_sb")
        p2w_dmas = [
            (wv_sb, WV[:]),
            (wp1_sb, WP1G[:]),
            (wp2_sb, WP2[:]),
            (bp1_sb, BP1E[:]),
        ]
        if fold_qk:
            m_sb = p2w.tile([128, 8, D], F32R, name="m_sb")
            wqb_sb = p2w.tile([128, 8, C], F32R, name="wqb_sb")
            p2w_dmas += [
                (m_sb, MQK[:]),
                (wqb_sb, WQB[:]),
            ]
        else:
            wq_sb = p2w.tile([128, 8, D], F32R, name="wq_sb")
            wk_sb = p2w.tile([128, 8, D], F32R, name="wk_sb")
            bast_sb = p2w.tile([128, 8, C], F32R, name="bast_sb")
            bq_sb = p2w.tile([128, 8], F32, name="bq_sb")
            bk_sb = p2w.tile([128, 8], F32, name="bk_sb")
            p2w_dmas += [
                (wq_sb, WQ[:]),
                (wk_sb, WK[:]),
                (bast_sb, BAST[:]),
                (bq_sb, BQ[:]),
                (bk_sb, BK[:]),
            ]

        # =================== PASS 1: embedding + LN ====================
        with (
            tc.tile_pool(name="p1w", bufs=1) as p1w,
            tc.tile_pool(name="p1x", bufs=2) as p1x,
            tc.tile_pool(name="p1n", bufs=1) as p1n,
            tc.tile_pool(name="p1o", bufs=2) as p1o,
            tc.tile_pool(name="p1s", bufs=4) as p1s,
            tc.tile_pool(name="ps1a", bufs=3, space="PSUM") as ps1a,
            tc.tile_pool(name="ps1b", bufs=2, space="PSUM") as ps1b,
        ):
            wemb_sb = p1w.tile([128, 8, D], F32R, name="wemb_sb")
            nc.sync.dma_start(
                wemb_sb[:], WEMB[:]
            )
            gs_sb = p1w.tile([128, 8], F32, name="gs_sb")
            bs_sb = p1w.tile([128, 8], F32, name="bs_sb")
            nc.sync.dma_start(gs_sb[:], GS[:])
            nc.sync.dma_start(bs_sb[:], BS[:])

            for b in range(NB):
                xb = p1x.tile([128, 8, C], F32R, name="xb", tag="xb")
                nc.sync.dma_start(
                    xb[:], X[b]
                )
                sen_n = p1n.tile([128, 4, D], F32, name="sen_n", tag="sen_n")
                for ck in range(4):
                    ps_s = ps1a.tile([128, D], F32, name="ps_s", tag="ps_s")
                    for dh in range(2):
                        for tk in range(8):
                            nc.tensor.matmul(
                                ps_s[:, dh * 512:(dh + 1) * 512],
                                xb[:, tk, ts(ck, 128)],
                                wemb_sb[:, tk, dh * 512:(dh + 1) * 512],
                                start=(tk == 0),
                                stop=(tk == 7 and not bemb_nz),
                            )
                        if bemb_nz:
                            nc.tensor.matmul(
                                ps_s[:, dh * 512:(dh + 1) * 512],
                                ones_r[0:1, :],
                                bemb_sb[0:1, dh * 512:(dh + 1) * 512],
                                start=False,
                                stop=True,
                            )
                    # LN stats over D (free dim)
                    bn6 = p1s.tile([128, 2, 6], F32, name="bn6", tag="st6")
                    nc.vector.bn_stats(bn6[:, 0, :], ps_s[:, 0:512])
                    nc.vector.bn_stats(bn6[:, 1, :], ps_s[:, 512:1024])
                    bnag = p1s.tile([128, 2], F32, name="bnag", tag="st2")
                    nc.vector.bn_aggr(bnag[:], bn6[:])
                    i_t = emit_rsqrt(p1s, bnag[:, 1:2], "l1")
                    negmi = p1s.tile([128, 1], F32, name="negmi", tag="st1c")
                    nc.vector.scalar_tensor_tensor(
                        negmi[:], bnag[:, 0:1], -1.0, i_t[:],
                        op0=ALU.mult, op1=ALU.mult,
                    )
                    nc.scalar.activation(
                        sen_n[:, ck, :], ps_s[:], AF.Identity,
                        bias=negmi[:], scale=i_t[:],
                    )
                # transpose + gamma/beta -> sensorT (f32r), stage to DRAM
                stout = p1o.tile([128, 8, C], F32R, name="stout", tag="stout")
                for dk in range(8):
                    ps_t = ps1b.tile([128, 512], F32, name="ps_t", tag="ps_t")
                    for ck in range(4):
                        nc.tensor.transpose(
                            ps_t[:, ts(ck, 128)], sen_n[:, ck, ts(dk, 128)],
                            id_sb[:],
                        )
                    nc.scalar.activation(
                        stout[:, dk, :], ps_t[:], AF.Identity,
                        bias=bs_sb[:, dk:dk + 1], scale=gs_sb[:, dk:dk + 1],
                    )
                nc.sync.dma_start(stg[b][:], stout[:])
                if 1 <= b <= len(p2w_dmas):
                    dst, src = p2w_dmas[b - 1]
                    nc.sync.dma_start(dst[:], src)
            for dst, src in p2w_dmas[max(0, NB - 1):]:
                nc.sync.dma_start(dst[:], src)

        # =================== PASS 2: attention + pooling ===============
        with (
            tc.tile_pool(name="big", bufs=5) as big,
            tc.tile_pool(name="a8", bufs=3) as a8,
            tc.tile_pool(name="p2m", bufs=1) as p2m,
            tc.tile_pool(name="p2s", bufs=2) as p2s,
            tc.tile_pool(name="ps2a", bufs=2, space="PSUM") as ps2a,
            tc.tile_pool(name="ps2b", bufs=2, space="PSUM") as ps2b,
        ):
            for b in range(NB):
                sT = big.tile([128, 8, C], F32R, name="sT", tag="big")
                nc.sync.dma_start(sT[:], stg[b][:])

                ab = a8.tile([128, 4, C], F32R, name="ab", tag="a8")
                sc = a8.tile([128, 4, C], F32R, name="sc", tag="a8")
                if fold_qk:
                    # RT = (S @ M)^T = M^T-as-lhsT @ S^T   [e(8 chunks), n=C]
                    RT = big.tile([128, 8, C], F32R, name="RT", tag="big")
                    for ec in range(8):
                        ptr = ps2a.tile([128, C], F32, name="ptr", tag="pq")
                        for kc in range(8):
                            nc.tensor.matmul(
                                ptr[:], m_sb[:, kc, ts(ec, 128)], sT[:, kc, :],
                                start=(kc == 0), stop=(kc == 7),
                            )
                        nc.vector.tensor_copy(RT[:, ec, :], ptr[:])
                    # age_bias = S @ (Wq basis^T) * isd  [n(4), m=C]
                    for nk in range(4):
                        pa = ps2a.tile([128, C], F32, name="pa", tag="pq")
                        for kc in range(8):
                            nc.tensor.matmul(
                                pa[:], sT[:, kc, ts(nk, 128)], wqb_sb[:, kc, :],
                                start=(kc == 0), stop=(kc == 7),
                            )
                        nc.vector.tensor_scalar(
                            ab[:, nk, :], pa[:], ISD, None, op0=ALU.mult
                        )
                    # scale = (R @ S^T) * isd + age_scale  [n(4), m=C]
                    for nk in range(4):
                        pa = ps2a.tile([128, C], F32, name="pa2", tag="pq")
                        for ec in range(8):
                            nc.tensor.matmul(
                                pa[:], RT[:, ec, ts(nk, 128)], sT[:, ec, :],
                                start=(ec == 0), stop=(ec == 7),
                            )
                        nc.vector.tensor_scalar(
                            sc[:, nk, :], pa[:], ISD, ages_t[:],
                            op0=ALU.mult, op1=ALU.add,
                        )
                else:
                    # QT, KT: [d'(8 chunks of 128), n=C]
                    QT = big.tile([128, 8, C], F32R, name="QT", tag="big")
                    KT = big.tile([128, 8, C], F32R, name="KT", tag="big")
                    for dst, w_sb, b_sb in ((QT, wq_sb, bq_sb), (KT, wk_sb, bk_sb)):
                        for dc in range(8):
                            pq = ps2a.tile([128, C], F32, name="pq", tag="pq")
                            for kc in range(8):
                                nc.tensor.matmul(
                                    pq[:], w_sb[:, kc, ts(dc, 128)], sT[:, kc, :],
                                    start=(kc == 0), stop=(kc == 7),
                                )
                            nc.scalar.activation(
                                dst[:, dc, :], pq[:], AF.Identity,
                                bias=b_sb[:, dc:dc + 1],
                            )
                    for nk in range(4):
                        pa = ps2a.tile([128, C], F32, name="pa", tag="pq")
                        for kc in range(8):
                            nc.tensor.matmul(
                                pa[:], QT[:, kc, ts(nk, 128)], bast_sb[:, kc, :],
                                start=(kc == 0), stop=(kc == 7),
                            )
                        nc.scalar.activation(ab[:, nk, :], pa[:], AF.Copy, scale=ISD)
                    for nk in range(4):
                        pa = ps2a.tile([128, C], F32, name="pa2", tag="pq")
                        for kc in range(8):
                            nc.tensor.matmul(
                                pa[:], QT[:, kc, ts(nk, 128)], KT[:, kc, :],
                                start=(kc == 0), stop=(kc == 7),
                            )
                        nc.scalar.activation(
                            sc[:, nk, :], pa[:], AF.Identity,
                            bias=ages_t[:], scale=ISD,
                        )

                # V: [m(4 chunks), d'=D] (before scores so sT dies early and
                # the next batch's sT prefetch DMA can start)
                V = big.tile([128, 4, D], F32R, name="V", tag="big")
                for mk in range(4):
                    for dh in range(2):
                        pv = ps2b.tile([128, 512], F32, name="pv", tag="pv", bufs=4)
                        for kc in range(8):
                            nc.tensor.matmul(
                                pv[:],
                                sT[:, kc, ts(mk, 128)],
                                wv_sb[:, kc, dh * 512:(dh + 1) * 512],
                                start=(kc == 0),
                                stop=(kc == 7 and not bv_nz),
                            )
                        if bv_nz:
                            nc.tensor.matmul(
                                pv[:],
                                ones_r[0:1, :],
                                bv_sb[0:1, dh * 512:(dh + 1) * 512],
                                start=False, stop=True,
                            )
                        nc.vector.tensor_copy(
                            V[:, mk, dh * 512:(dh + 1) * 512], pv[:]
                        )

                # scores = age_bias^T @ scale, softmax over free dim
                exp_s = a8.tile([128, 4, C], F32, name="exp_s", tag="a8")
                recips = p2s.tile([128, 4], F32, name="recips", tag="rec")
                bn6f = p2s.tile([128, 4, 2, 6], F32, name="bn6f", tag="bn6f")
                bnagf = p2s.tile([128, 4, 2], F32, name="bnagf", tag="bnagf")
                for nk in range(4):
                    psc = ps2a.tile([128, C], F32, name="psc", tag="pq")
                    for jk in range(4):
                        nc.tensor.matmul(
                            psc[:], ab[:, jk, ts(nk, 128)], sc[:, jk, :],
                            start=(jk == 0), stop=(jk == 3),
                        )
                    negmax = p2s.tile([128, 1], F32, name="negmax", tag="nmx")
                    nc.vector.tensor_reduce(
                        negmax[:], psc[:], axis=AX.X, op=ALU.max, negate=True
                    )
                    sume = p2s.tile([128, 1], F32, name="sume", tag="sme")
                    nc.scalar.activation(
                        exp_s[:, nk, :], psc[:], AF.Exp,
                        bias=negmax[:], accum_out=sume[:],
                    )
                    nc.vector.reciprocal(recips[:, nk:nk + 1], sume[:])

                # transpose exp_s -> exp_sT [m(4), n=C] (f32r)
                expT = a8.tile([128, 4, C], F32R, name="expT", tag="a8")
                for mk in range(4):
                    ptx = ps2a.tile([128, C], F32, name="ptx", tag="pq")
                    for nk in range(4):
                        nc.tensor.transpose(
                            ptx[:, ts(nk, 128)], exp_s[:, nk, ts(mk, 128)],
                            id_sb[:],
                        )
                    nc.vector.tensor_copy(expT[:, mk, :], ptx[:])

                # fused = softmax(scores) @ V * isd, then LN (no gamma/beta)
                fN = big.tile([128, 4, D], F32R, name="fN", tag="big")
                for nk in range(4):
                    pfs = []
                    for dh in range(2):
                        pf = ps2b.tile([128, 512], F32, name="pf", tag="pv", bufs=4)
                        for mk in range(4):
                            nc.tensor.matmul(
                                pf[:],
                                expT[:, mk, ts(nk, 128)],
                                V[:, mk, dh * 512:(dh + 1) * 512],
                                start=(mk == 0), stop=(mk == 3),
                            )
                        nc.vector.bn_stats(bn6f[:, nk, dh, :], pf[:])
                        pfs.append(pf)
                    nc.vector.bn_aggr(bnagf[:, nk, :], bn6f[:, nk, :, :])
                    # s = recip * isd ; scale_eff = s*i ; bias_eff = -m*s*i
                    s_t = p2s.tile([128, 1], F32, name="s_t", tag="s_t")
                    nc.vector.tensor_scalar(
                        s_t[:], recips[:, nk:nk + 1], ISD, None, op0=ALU.mult
                    )
                    s2_t = p2s.tile([128, 1], F32, name="s2_t", tag="s2_t")
                    nc.vector.tensor_tensor(s2_t[:], s_t[:], s_t[:], op=ALU.mult)
                    vs_t = p2s.tile([128, 1], F32, name="vs_t", tag="vs_t")
                    nc.vector.tensor_tensor(
                        vs_t[:], bnagf[:, nk, 1:2], s2_t[:], op=ALU.mult
                    )
                    i2_t = emit_rsqrt(p2s, vs_t[:], "l2")
                    se_t = p2s.tile([128, 1], F32, name="se_t", tag="se_t")
                    nc.vector.tensor_tensor(se_t[:], s_t[:], i2_t[:], op=ALU.mult)
                    be_t = p2s.tile([128, 1], F32, name="be_t", tag="be_t")
                    nc.vector.scalar_tensor_tensor(
                        be_t[:], bnagf[:, nk, 0:1], -1.0, se_t[:],
                        op0=ALU.mult, op1=ALU.mult,
                    )
                    for dh in range(2):
                        nc.scalar.activation(
                            fN[:, nk, dh * 512:(dh + 1) * 512], pfs[dh][:],
                            AF.Identity, bias=be_t[:], scale=se_t[:],
                        )

                # fNT = fN^T [d(8 chunks), n=C] (f32r; transpose reads f32r
                # bits as f32 — value-preserving since f32r is fp32 with a
                # truncated mantissa)
                fNT = big.tile([128, 8, C], F32R, name="fNT", tag="big")
                for dk in range(8):
                    ptf = ps2a.tile([128, C], F32, name="ptf", tag="pq")
                    for nk in range(4):
                        nc.tensor.transpose(
                            ptf[:, ts(nk, 128)],
                            fN[:, nk, ts(dk, 128)].bitcast(F32),
                            id_sb[:],
                        )
                    nc.vector.tensor_copy(fNT[:, dk, :], ptf[:])

                # attention pooling: hT = gelu(Wp1g^T @ fNT + bp1e)
                ph = ps2a.tile([128, C], F32, name="ph", tag="pq")
                for kc in range(8):
                    nc.tensor.matmul(
                        ph[:], wp1_sb[:, kc, :], fNT[:, kc, :],
                        start=(kc == 0), stop=(kc == 7),
                    )
                # gelu via tanh formula (Square/Tanh stay in the loaded ACT
                # table set -- avoids 2 ACT_TABLE_LOADs per batch)
                gx = p2m.tile([H, C], F32, name="gx", tag="gx")
                nc.scalar.activation(gx[:], ph[0:H, :], AF.Identity, bias=bp1_sb[:])
                g2 = p2m.tile([H, C], F32, name="g2", tag="g2")
                nc.scalar.activation(g2[:], gx[:], AF.Square)
                nc.vector.tensor_scalar(
                    g2[:], g2[:], 0.044715 * 0.7978845608028654,
                    0.7978845608028654, op0=ALU.mult, op1=ALU.add,
                )
                g3 = p2m.tile([H, C], F32, name="g3", tag="g3")
                nc.vector.tensor_tensor(g3[:], g2[:], gx[:], op=ALU.mult)
                nc.scalar.activation(g3[:], g3[:], AF.Tanh)
                nc.vector.tensor_scalar(g3[:], g3[:], 1.0, None, op0=ALU.add)
                hT = p2m.tile([H, C], BF16, name="hT", tag="hT")
                nc.vector.scalar_tensor_tensor(
                    hT[:], g3[:], 0.5, gx[:], op0=ALU.mult, op1=ALU.mult,
                )

                # pool_scores row [1, C] + softmax
                pps = ps2a.tile([1, C], F32, name="pps", tag="pq")
                nc.tensor.matmul(pps[:], wp2_sb[:], hT[:], start=True, stop=True)
                pnm = p2s.tile([1, 1], F32, name="pnm", tag="pnm")
                nc.vector.tensor_reduce(
                    pnm[:], pps[:], axis=AX.X, op=ALU.max, negate=True
                )
                pw = p2m.tile([1, C], F32, name="pw", tag="row")
                pse = p2s.tile([1, 1], F32, name="pse", tag="pse")
                nc.scalar.activation(
                    pw[:], pps[:], AF.Exp, bias=pnm[:], accum_out=pse[:]
                )
                prc = p2s.tile([1, 1], F32, name="prc", tag="prc")
                nc.vector.reciprocal(prc[:], pse[:])

                # normalized pool weights as columns [n(4 chunks), 2] (f32r)
                pwn = p2m.tile([1, C], F32, name="pwn", tag="rown")
                nc.vector.tensor_scalar(
                    pwn[:], pw[:], prc[0:1, 0:1], None, op0=ALU.mult
                )
                ppw = ps2b.tile([128, 4], F32, name="ppw", tag="pwt", bufs=1)
                for nk in range(4):
                    nc.tensor.transpose(
                        ppw[:, nk:nk + 1], pwn[0:1, ts(nk, 128)], id_sb[0:1, 0:1]
                    )
                pwc = p2m.tile([128, 4, 2], F32R, name="pwc", tag="pwc")
                nc.scalar.activation(pwc[:, :, 0], ppw[:, 0:4], AF.Copy)
                nc.scalar.activation(pwc[:, :, 1], ppw[:, 0:4], AF.Copy)

                # pooled columns: pooledT[:,dk,b] = gf*(fN^T @ pw_n) + bf
                for dk in range(8):
                    pp = ps2b.tile([128, 2], F32, name="pp", tag="pp", bufs=1)
                    for nk in range(4):
                        nc.tensor.matmul(
                            pp[:], fN[:, nk, ts(dk, 128)], pwc[:, nk, :],
                            start=(nk == 0), stop=(nk == 3),
                        )
                    nc.scalar.activation(
                        pooledT[:, dk, b:b + 1], pp[:, 0:1], AF.Identity,
                        bias=bf_sb[:, dk:dk + 1], scale=gf_sb[:, dk:dk + 1],
                    )

        p2w.release()

        # =================== PASS 3: fc head ===========================
        with (
            tc.tile_pool(name="p3w", bufs=3) as p3w,
            tc.tile_pool(name="p3m", bufs=1) as p3m,
            tc.tile_pool(name="p3s", bufs=8) as p3s,
            tc.tile_pool(name="ps3", bufs=1, space="PSUM") as ps3,
            tc.tile_pool(name="ps3r", bufs=1, space="PSUM") as ps3r,
        ):
            bf1_sb = p3m.tile([128, 16], F32, name="bf1_sb")
            bf2_sb = p3m.tile([128, 16], F32, name="bf2_sb")
            wf3_sb = p3m.tile([128, 16, 2], F32R, name="wf3_sb")
            nc.sync.dma_start(bf1_sb[:], BF1[:])
            nc.sync.dma_start(bf2_sb[:], BF2[:])
            nc.sync.dma_start(wf3_sb[:], WF3C[:])
            h1T = p3m.tile([128, 16, NB], F32R, name="h1T")
            h2T = p3m.tile([128, 16, NB], F32R, name="h2T")

            # h1T = leaky_relu(Wf1^T @ pooledT + bf1)
            for g in range(4):
                pg = [
                    ps3.tile([128, NB], F32, name=f"pg{g}_{j}", tag=f"pg{j}")
                    for j in range(4)
                ]
                for kd in range(8):
                    wt = p3w.tile([128, 512], F32R, name="wt1", tag="w3")
                    nc.sync.dma_start(wt[:], WF1[g * 8 + kd])
                    for j in range(4):
                        nc.tensor.matmul(
                            pg[j][:], wt[:, ts(j, 128)], pooledT[:, kd, :],
                            start=(kd == 0), stop=(kd == 7),
                        )
                for j in range(4):
                    mf = g * 4 + j
                    nc.scalar.activation(
                        h1T[:, mf, :], pg[j][:], AF_LRELU,
                        bias=bf1_sb[:, mf:mf + 1], alpha=0.01,
                    )

            # h2T = Wf2^T @ h1T + bf2
            for g in range(4):
                pg = [
                    ps3.tile([128, NB], F32, name=f"qg{g}_{j}", tag=f"pg{j}")
                    for j in range(4)
                ]
                for kf in range(16):
                    wt = p3w.tile([128, 512], F32R, name="wt2", tag="w3")
                    nc.sync.dma_start(wt[:], WF2[g * 16 + kf])
                    for j in range(4):
                        nc.tensor.matmul(
                            pg[j][:], wt[:, ts(j, 128)], h1T[:, kf, :],
                            start=(kf == 0), stop=(kf == 15),
                        )
                for j in range(4):
                    mf = g * 4 + j
                    nc.scalar.activation(
                        h2T[:, mf, :], pg[j][:], AF.Identity,
                        bias=bf2_sb[:, mf:mf + 1],
                    )

            # rul = abs(h2 @ Wf3 + bf3)
            prul = ps3r.tile([NB, 2], F32, name="prul")
            for k in range(16):
                nc.tensor.matmul(
                    prul[:], h2T[:, k, :], wf3_sb[:, k, :],
                    start=(k == 0), stop=(k == 15),
                )
            bf3_t = p3s.tile([NB, 1], F32, name="bf3_t")
            nc.gpsimd.memset(bf3_t[:], bf3_f)
            rul_sb = p3s.tile([NB, 1], F32, name="rul_sb")
            nc.scalar.activation(rul_sb[:], prul[:, 0:1], AF.Abs, bias=bf3_t[:])
            nc.sync.dma_start(RUL[:], rul_sb[:])

        glob.release()
        stg_pool.release()

    nc.compile()
    return nc


def _prep_in_maps(inputs):
    x_enc = np.asarray(inputs["x_enc"], dtype=np.float32)
    W_emb = np.asarray(inputs["W_emb"], dtype=np.float32)
    b_emb = np.asarray(inputs["b_emb"], dtype=np.float32)
    g_s = np.asarray(inputs["g_s"], dtype=np.float32)
    b_s = np.asarray(inputs["b_s"], dtype=np.float32)
    basis = np.asarray(inputs["basis"], dtype=np.float32)
    Wq = np.asarray(inputs["Wq"], dtype=np.float32)
    bq = np.asarray(inputs["bq"], dtype=np.float32)
    Wk = np.asarray(inputs["Wk"], dtype=np.float32)
    bk = np.asarray(inputs["bk"], dtype=np.float32)
    Wv = np.asarray(inputs["Wv"], dtype=np.float32)
    bv = np.asarray(inputs["bv"], dtype=np.float32)
    g_f = np.asarray(inputs["g_f"], dtype=np.float32)
    b_f = np.asarray(inputs["b_f"], dtype=np.float32)
    Wp1 = np.asarray(inputs["Wp1"], dtype=np.float32)
    bp1 = np.asarray(inputs["bp1"], dtype=np.float32)
    Wp2 = np.asarray(inputs["Wp2"], dtype=np.float32)
    Wf1 = np.asarray(inputs["Wf1"], dtype=np.float32)
    bf1 = np.asarray(inputs["bf1"], dtype=np.float32)
    Wf2 = np.asarray(inputs["Wf2"], dtype=np.float32)
    bf2 = np.asarray(inputs["bf2"], dtype=np.float32)
    Wf3 = np.asarray(inputs["Wf3"], dtype=np.float32)

    wp1g = np.zeros((D, 128), np.float32)
    wp1g[:, :H] = g_f[:, None] * Wp1
    bp1e = (b_f @ Wp1 + bp1).reshape(H, 1).astype(np.float32)

    import ml_dtypes

    fold_qk = not (np.any(bq) or np.any(bk))
    common = {
        "wemb": _round_f32r(_sbuf_layout(W_emb, 8)),
        "wv": _round_f32r(_sbuf_layout(Wv, 8)),
        "wp1g": _round_f32r(_sbuf_layout(wp1g, 8)),
        "wp2": Wp2.astype(ml_dtypes.bfloat16),
        "wf1": _round_f32r(
            Wf1.reshape(8, 128, 4, 512).transpose(2, 0, 1, 3).reshape(32, 128, 512)
        ),
        "wf2": _round_f32r(
            Wf2.reshape(16, 128, 4, 512).transpose(2, 0, 1, 3).reshape(64, 128, 512)
        ),
        "wf3c": _round_f32r(
            np.repeat(_chunk_major(Wf3[:, 0], 16)[:, :, None], 2, axis=2)
        ),
        "gs_c": _chunk_major(g_s, 8),
        "bs_c": _chunk_major(b_s, 8),
        "gf_c": _chunk_major(g_f, 8),
        "bf_c": _chunk_major(b_f, 8),
        "bp1e": bp1e,
        "bf1_c": _chunk_major(bf1, 16),
        "bf2_c": _chunk_major(bf2, 16),
        "ident": np.eye(128, dtype=np.float32),
    }
    if fold_qk:
        common["m_mat"] = _round_f32r(_sbuf_layout(
            (Wq.astype(np.float64) @ Wk.astype(np.float64).T).astype(np.float32),
            8,
        ))
        common["wqb"] = _round_f32r(_sbuf_layout(
            (Wq.astype(np.float64) @ basis.astype(np.float64).T).astype(np.float32),
            8,
        ))
    else:
        common["wq"] = _round_f32r(_sbuf_layout(Wq, 8))
        common["wk"] = _round_f32r(_sbuf_layout(Wk, 8))
        common["bast"] = _round_f32r(_sbuf_layout(np.ascontiguousarray(basis.T), 8))
        common["bq_c"] = _chunk_major(bq, 8)
        common["bk_c"] = _chunk_major(bk, 8)
    bemb_nz = bool(np.any(b_emb))
    bv_nz = bool(np.any(bv))
    if bemb_nz:
        common["bemb_row"] = _round_f32r(b_emb.reshape(1, D))
    if bv_nz:
        common["bv_row"] = _round_f32r(bv.reshape(1, D))

    in_maps = []
    for c in range(NCORES):
        m = dict(common)
        xs = x_enc[c * NB:(c + 1) * NB]
        m["x"] = _round_f32r(
            xs.reshape(NB, 8, 128, C).transpose(0, 2, 1, 3).reshape(NB, 128, 8 * C)
        )
        in_maps.append(m)

    age_scale_f = float(np.asarray(inputs["age_scale"], dtype=np.float32))
    bf3_f = float(np.asarray(inputs["bf3"], dtype=np.float32).reshape(-1)[0])
    return in_maps, age_scale_f, bf3_f, bemb_nz, bv_nz, fold_qk


_NC_CACHE = {}


def build_program(inputs):
    in_maps, age_scale_f, bf3_f, bemb_nz, bv_nz, fold_qk = _prep_in_maps(inputs)
    key = (age_scale_f, bf3_f, bemb_nz, bv_nz, fold_qk)
    if key not in _NC_CACHE:
        _NC_CACHE[key] = _build(age_scale_f, bf3_f, bemb_nz, bv_nz, fold_qk)
    return _NC_CACHE[key], in_maps


def kernel(**inputs):
    from concourse.bass_utils import run_bass_kernel_spmd

    nc, in_maps = build_program(inputs)
    res = run_bass_kernel_spmd(nc, in_maps, core_ids=list(range(NCORES)))
    out = np.concatenate(
        [res.results[c]["rul"] for c in range(NCORES)], axis=0
    ).astype(np.float32)
    return out



# revision 3
# speedup vs baseline: 1.3178x; 1.3178x over previous
"""Trainium2 Bass kernel for nn_CLIP_69458211111620 (v2: fused pipeline).

Data-parallel over batch B=128 across 8 NeuronCores (16 batches/core).
Single fused pass per batch (no DRAM staging), software-pipelined 4 deep:
  P1(i): emb matmuls + LN + transpose -> S
  P2(i-1): RT/ab/sc/V/scoresT matmuls, exp (pre-transposed softmax), fused, LN
  P3(i-2): fNT transpose, pooling MLP hT
  P4(i-3): pool softmax + pooled columns
then fc head on SBUF-prefetched weights.

Precision: fp8 e4m3 + DoubleRow for big matmuls where the error budget
allows (sim.py ablations), bf16 elsewhere, f32r where critical.
"""
import sys

sys.path.insert(0, "/opt/trn_rl_repo")

import numpy as np
import ml_dtypes

NCORES = 8
NB = 16          # batches per core
T, C, D, DF, H = 1024, 512, 1024, 2048, 64
ISD = 1.0 / 32.0  # 1/sqrt(D)
EPS = 1e-5

# ---- precision config (validated by sim.py ablations) -----------------
# fp8 e4m3 on any attention-chain operand exceeds the 2e-2 budget (the
# softmax sharply amplifies quantization noise); all-bf16 sims at 6.6e-3.
EMB_FP8 = False   # x, wemb e4m3 (DoubleRow)
ATTN_FP8 = False  # S, M, wqb e4m3 -> RT/ab/sc DoubleRow
V_FP8 = False     # wv e4m3 (uses e4m3 S) -> V matmul DoubleRow
ABSC_DT = "bf16"  # scores matmul operand dtype ("bf16" | "f32r")
HEAD_DT = "bf16"  # fc head weight dtype ("bf16" | "f32r")

S_EMB = 64.0 if EMB_FP8 else 1.0
S_M = 16.0 if ATTN_FP8 else 1.0
S_WQB = 16.0 if ATTN_FP8 else 1.0
S_WV = 32.0 if V_FP8 else 1.0

E4NP = ml_dtypes.float8_e4m3
BFNP = ml_dtypes.bfloat16


def _round_f32r(x):
    u = np.ascontiguousarray(x, dtype=np.float32).view(np.uint32).copy()
    lsb = (u >> np.uint32(12)) & np.uint32(1)
    u += np.uint32(0x7FF) + lsb
    u &= np.uint32(0xFFFFF000)
    return u.view(np.float32)


def _chunk_major(v, nchunk):
    return np.ascontiguousarray(
        np.asarray(v, dtype=np.float32).reshape(nchunk, 128).T
    )


def _sbuf_layout(w, nk):
    """[nk*128, F] -> [128, nk, F] partition-major image."""
    w = np.asarray(w, dtype=np.float32)
    f = w.shape[1]
    return np.ascontiguousarray(
        w.reshape(nk, 128, f).transpose(1, 0, 2)
    )


def _cast(x, dt):
    if dt == "e4m3":
        return np.clip(x, -240.0, 240.0).astype(E4NP)
    if dt == "bf16":
        return np.asarray(x, np.float32).astype(BFNP)
    return _round_f32r(x)


def _build(age_scale_f, bf3_f, bemb_nz, bv_nz, sim_acts=False):
    import concourse.tile as tile
    import concourse.bass as bass
    from concourse import bacc, mybir

    F32 = mybir.dt.float32
    F32R = mybir.dt.float32r
    BF16 = mybir.dt.bfloat16
    E4M3 = mybir.dt.float8e4
    AF = mybir.ActivationFunctionType
    ALU = mybir.AluOpType
    AX = mybir.AxisListType
    PM = mybir.MatmulPerfMode
    ts = bass.ts
    AF_LRELU = AF.Relu if sim_acts else AF.Lrelu

    I32 = mybir.dt.int32
    RSQRT_MAGIC = 0x5F3759DF

    SDT = E4M3 if (ATTN_FP8 or V_FP8) else BF16
    XDT = E4M3 if EMB_FP8 else BF16
    WEDT = E4M3 if EMB_FP8 else BF16
    MDT = E4M3 if ATTN_FP8 else BF16
    WVDT = E4M3 if V_FP8 else BF16
    ABDT = F32R if ABSC_DT == "f32r" else BF16
    HDT = F32R if HEAD_DT == "f32r" else BF16

    def kch(n, fp8):
        """Chunk iteration: DoubleRow pairs if fp8 else single chunks."""
        if fp8:
            return [
                (slice(2 * p, 2 * p + 2), p == 0, p == n // 2 - 1, PM.DoubleRow)
                for p in range(n // 2)
            ]
        return [(slice(k, k + 1), k == 0, k == n - 1, None) for k in range(n)]

    nc = bacc.Bacc("TRN2", target_bir_lowering=False, debug=False)

    def inp(name, shape, dt):
        return nc.dram_tensor(name, shape, dt, kind="ExternalInput").ap()

    X = inp("x", (NB, 128, 8, C), XDT)
    WEMB = inp("wemb", (128, 8, D), WEDT)
    MQK = inp("m_mat", (128, 8, D), MDT)
    WQB = inp("wqb", (128, 8, C), MDT)
    WV = inp("wv", (128, 8, D), WVDT)
    WP1G = inp("wp1g", (128, 8, 128), BF16)
    WP2 = inp("wp2", (H, 1), BF16)
    WF1 = inp("wf1", (32, 128, 512), HDT)
    WF2 = inp("wf2", (64, 128, 512), HDT)
    WF3C = inp("wf3c", (128, DF // 128, 2), F32R)
    GS = inp("gs_c", (128, 8), F32)
    BS = inp("bs_c", (128, 8), F32)
    GF = inp("gf_c", (128, 8), F32)
    BF_ = inp("bf_c", (128, 8), F32)
    BP1E = inp("bp1e", (H, 1), F32)
    BF1 = inp("bf1_c", (128, 16), F32)
    BF2 = inp("bf2_c", (128, 16), F32)
    IDENT = inp("ident", (128, 128), BF16)
    BEMB = inp("bemb_row", (1, D), F32R) if bemb_nz else None
    BVR = inp("bv_row", (1, D), F32R) if bv_nz else None
    RUL = nc.dram_tensor("rul", (NB, 1), F32, kind="ExternalOutput").ap()

    with tile.TileContext(nc) as tc:
        # ---- long-lived tiles ----------------------------------------
        glob = tc.alloc_tile_pool(name="glob", bufs=1)
        id_sb = glob.tile([128, 128], BF16, name="id_sb")
        magic_t = glob.tile([128, 4], I32, name="magic_t")
        ages_t = glob.tile([128, 1], F32, name="ages_t")
        ones_c = glob.tile([128, 1], BF16, name="ones_c")
        idf1 = glob.tile([1, 1], F32, name="idf1")
        pooledT = glob.tile([128, 8, NB], BF16, name="pooledT")
        gs_sb = glob.tile([128, 8], F32, name="gs_sb")
        bs_sb = glob.tile([128, 8], F32, name="bs_sb")
        gf_sb = glob.tile([128, 8], F32, name="gf_sb")
        bf_sb = glob.tile([128, 8], F32, name="bf_sb")
        nc.sync.dma_start(id_sb[:], IDENT[:])
        nc.sync.dma_start(gs_sb[:], GS[:])
        nc.sync.dma_start(bs_sb[:], BS[:])
        nc.sync.dma_start(gf_sb[:], GF[:])
        nc.sync.dma_start(bf_sb[:], BF_[:])
        nc.gpsimd.memset(magic_t[:], RSQRT_MAGIC)
        nc.gpsimd.memset(ages_t[:], age_scale_f)
        nc.gpsimd.memset(ones_c[:], 1.0)
        nc.gpsimd.memset(idf1[:], 1.0)
        ones_r = None
        if bemb_nz or bv_nz:
            ones_r = glob.tile([1, 128], F32R, name="ones_r")
            nc.gpsimd.memset(ones_r[:], 1.0)
        bemb_sb = None
        if bemb_nz:
            bemb_sb = glob.tile([1, D], F32R, name="bemb_sb")
            nc.sync.dma_start(bemb_sb[:], BEMB[:])
        bv_sb = None
        if bv_nz:
            bv_sb = glob.tile([1, D], F32R, name="bv_sb")
            nc.sync.dma_start(bv_sb[:], BVR[:])

        # ---- weights --------------------------------------------------
        wts = tc.alloc_tile_pool(name="wts", bufs=1)
        wemb_sb = wts.tile([128, 8, D], WEDT, name="wemb_sb")
        m_sb = wts.tile([128, 8, D], MDT, name="m_sb")
        wqb_sb = wts.tile([128, 8, C], MDT, name="wqb_sb")
        wv_sb = wts.tile([128, 8, D], WVDT, name="wv_sb")
        wp1_sb = wts.tile([128, 8, 128], BF16, name="wp1_sb")
        wp2_sb = wts.tile([H, 1], BF16, name="wp2_sb")
        bp1_sb = wts.tile([H, 1], F32, name="bp1_sb")
        bf1_sb = wts.tile([128, 16], F32, name="bf1_sb")
        bf2_sb = wts.tile([128, 16], F32, name="bf2_sb")
        wf3_sb = wts.tile([128, 16, 2], F32R, name="wf3_sb")
        nc.sync.dma_start(wemb_sb[:], WEMB[:])
        nc.sync.dma_start(m_sb[:], MQK[:])
        nc.sync.dma_start(wqb_sb[:], WQB[:])
        nc.sync.dma_start(wv_sb[:], WV[:])
        nc.sync.dma_start(wp1_sb[:], WP1G[:])
        nc.sync.dma_start(wp2_sb[:], WP2[:])
        nc.sync.dma_start(bp1_sb[:], BP1E[:])
        nc.sync.dma_start(bf1_sb[:], BF1[:])
        nc.sync.dma_start(bf2_sb[:], BF2[:])
        nc.sync.dma_start(wf3_sb[:], WF3C[:])

        def emit_rsqrt(pool, v_ap, w, tagp, eps, iters=2):
            """[128,w] -> 1/sqrt(v + eps) elementwise on DVE (Quake+Newton)."""
            ve = pool.tile([128, w], F32, name=f"{tagp}ve", tag=f"{tagp}ve")
            nc.vector.tensor_scalar(ve[:], v_ap, eps, None, op0=ALU.add)
            y = pool.tile([128, w], F32, name=f"{tagp}y0", tag=f"{tagp}y0")
            nc.vector.tensor_scalar(
                y.bitcast(I32)[:], ve.bitcast(I32)[:], 1, None,
                op0=ALU.logical_shift_right,
            )
            nc.vector.scalar_tensor_tensor(
                y.bitcast(I32)[:], y.bitcast(I32)[:], -1, magic_t[:, 0:w],
                op0=ALU.mult, op1=ALU.add,
            )
            for it in range(iters):
                a = pool.tile([128, w], F32, name=f"{tagp}a{it}", tag=f"{tagp}a{it}")
                nc.vector.tensor_tensor(a[:], y[:], y[:], op=ALU.mult)
                nc.vector.tensor_tensor(a[:], a[:], ve[:], op=ALU.mult)
                nc.vector.tensor_scalar(
                    a[:], a[:], -0.5, 1.5, op0=ALU.mult, op1=ALU.add
                )
                nc.vector.tensor_tensor(y[:], y[:], a[:], op=ALU.mult)
            return y

        # ---- pipelined main loop -------------------------------------
        with (
            tc.tile_pool(name="px", bufs=2) as px,
            tc.tile_pool(name="pw3", bufs=16) as pw3,
            tc.tile_pool(name="psen", bufs=1) as psen,
            tc.tile_pool(name="pS", bufs=2) as pS,
            tc.tile_pool(name="pmid", bufs=1) as pmid,
            tc.tile_pool(name="pfn", bufs=3) as pfn,
            tc.tile_pool(name="psc1", bufs=2) as psc1,
            tc.tile_pool(name="psc2", bufs=2) as psc2,
            tc.tile_pool(name="pgel", bufs=1) as pgel,
            tc.tile_pool(name="ps_emb", bufs=3, space="PSUM") as ps_emb,
            tc.tile_pool(name="ps_main", bufs=3, space="PSUM") as ps_main,
            tc.tile_pool(name="ps_small", bufs=2, space="PSUM") as ps_small,
        ):
            st = [dict() for _ in range(NB)]

            def p1_emb(i):
                s = st[i]
                if i == 0:
                    xb = px.tile([128, 8, C], XDT, name="xb", tag="xb")
                    nc.sync.dma_start(xb[:], X[0])
                    s["xb"] = xb
                xb = s.pop("xb")
                if i + 1 < NB:
                    xb2 = px.tile([128, 8, C], XDT, name="xb2", tag="xb")
                    nc.sync.dma_start(xb2[:], X[i + 1])
                    st[i + 1]["xb"] = xb2
                sen_n = psen.tile([128, 4, D], BF16, name="sen_n", tag="sen")
                for ck in range(4):
                    bn6 = psc1.tile([128, 2, 6], F32, name="bn6", tag="st6")
                    ph2 = []
                    for dh in range(2):
                        ps_s = ps_emb.tile([128, 512], F32, name="ps_s", tag="ps_s")
                        for sl, sta, stp, pm in kch(8, EMB_FP8):
                            nc.tensor.matmul(
                                ps_s[:],
                                xb[:, sl, ts(ck, 128)],
                                wemb_sb[:, sl, dh * 512:(dh + 1) * 512],
                                start=sta,
                                stop=(stp and not bemb_nz),
                                perf_mode=pm,
                            )
                        if bemb_nz:
                            nc.tensor.matmul(
                                ps_s[:],
                                ones_r[0:1, :],
                                bemb_sb[0:1, dh * 512:(dh + 1) * 512],
                                start=False, stop=True,
                            )
                        nc.vector.bn_stats(bn6[:, dh, :], ps_s[:])
                        ph2.append(ps_s)
                    bnag = psc1.tile([128, 2], F32, name="bnag", tag="bnag")
                    nc.vector.bn_aggr(bnag[:], bn6[:])
                    i_t = emit_rsqrt(
                        psc1, bnag[:, 1:2], 1, "l1", EPS * S_EMB * S_EMB
                    )
                    negmi = psc1.tile([128, 1], F32, name="negmi", tag="negmi")
                    nc.vector.scalar_tensor_tensor(
                        negmi[:], bnag[:, 0:1], -1.0, i_t[:],
                        op0=ALU.mult, op1=ALU.mult,
                    )
                    for dh in range(2):
                        nc.scalar.activation(
                            sen_n[:, ck, dh * 512:(dh + 1) * 512],
                            ph2[dh][:], AF.Identity,
                            bias=negmi[:], scale=i_t[:],
                        )
                s["sen_n"] = sen_n

            def p1_tr(i):
                s = st[i]
                sen_n = s.pop("sen_n")
                S_t = pS.tile([128, 8, C], SDT, name="S_t", tag="S")
                for dk in range(8):
                    ps_t = ps_small.tile([128, 512], BF16, name="ps_t", tag="sm")
                    for ck in range(4):
                        nc.tensor.transpose(
                            ps_t[:, ts(ck, 128)], sen_n[:, ck, ts(dk, 128)],
                            id_sb[:],
                        )
                    nc.scalar.activation(
                        S_t[:, dk, :], ps_t[:], AF.Identity,
                        bias=bs_sb[:, dk:dk + 1], scale=gs_sb[:, dk:dk + 1],
                    )
                s["S"] = S_t

            def p2_big(i):
                s = st[i]
                S_t = s.pop("S")
                # RT = (S M)^T  [e(8), n=C]
                RT = pmid.tile([128, 8, C], MDT, name="RT", tag="RT")
                for ec in range(8):
                    ptr = ps_main.tile([128, C], F32, name="ptr", tag="pm")
                    for sl, sta, stp, pm in kch(8, ATTN_FP8):
                        nc.tensor.matmul(
                            ptr[:], m_sb[:, sl, ts(ec, 128)], S_t[:, sl, :],
                            start=sta, stop=stp, perf_mode=pm,
                        )
                    nc.scalar.activation(RT[:, ec, :], ptr[:], AF.Copy)
                # ab = S Wqb * isd/s_wqb  [n(4), m=C]
                ab = pmid.tile([128, 4, C], ABDT, name="ab", tag="ab")
                for nk in range(4):
                    pa = ps_main.tile([128, C], F32, name="pa", tag="pm")
                    for sl, sta, stp, pm in kch(8, ATTN_FP8):
                        nc.tensor.matmul(
                            pa[:], S_t[:, sl, ts(nk, 128)], wqb_sb[:, sl, :],
                            start=sta, stop=stp, perf_mode=pm,
                        )
                    nc.scalar.activation(
                        ab[:, nk, :], pa[:], AF.Copy, scale=float(ISD / S_WQB)
                    )
                # sc = R S^T * isd/s_M + age  [n(4), m=C]
                sc = pmid.tile([128, 4, C], ABDT, name="sc", tag="sc")
                for nk in range(4):
                    pa = ps_main.tile([128, C], F32, name="pa2", tag="pm")
                    for sl, sta, stp, pm in kch(8, ATTN_FP8):
                        nc.tensor.matmul(
                            pa[:], RT[:, sl, ts(nk, 128)], S_t[:, sl, :],
                            start=sta, stop=stp, perf_mode=pm,
                        )
                    nc.scalar.activation(
                        sc[:, nk, :], pa[:], AF.Identity,
                        bias=ages_t[:], scale=float(ISD / S_M),
                    )
                # V = S Wv  [m(4), D]
                V = pmid.tile([128, 4, D], BF16, name="V", tag="V")
                for mk in range(4):
                    for dh in range(2):
                        pv = ps_main.tile([128, 512], F32, name="pv", tag="pm")
                        for sl, sta, stp, pm in kch(8, V_FP8):
                            nc.tensor.matmul(
                                pv[:],
                                S_t[:, sl, ts(mk, 128)],
                                wv_sb[:, sl, dh * 512:(dh + 1) * 512],
                                start=sta,
                                stop=(stp and not bv_nz),
                                perf_mode=pm,
                            )
                        if bv_nz:
                            nc.tensor.matmul(
                                pv[:],
                                ones_r[0:1, :],
                                bv_sb[0:1, dh * 512:(dh + 1) * 512],
                                start=False, stop=True,
                            )
                        nc.vector.tensor_copy(
                            V[:, mk, dh * 512:(dh + 1) * 512], pv[:]
                        )
                s["V"] = V
                # scoresT[k,n] = sum_j sc[j,k] ab[j,n]; exp -> expT (bf16)
                expT = pmid.tile([128, 4, C], BF16, name="expT", tag="expT")
                for kk in range(4):
                    psc = ps_main.tile([128, C], F32, name="psc", tag="pm")
                    for jk in range(4):
                        nc.tensor.matmul(
                            psc[:], sc[:, jk, ts(kk, 128)], ab[:, jk, :],
                            start=(jk == 0), stop=(jk == 3),
                        )
                    nc.scalar.activation(expT[:, kk, :], psc[:], AF.Exp)
                s["expT"] = expT

            def p2_fused(i):
                s = st[i]
                V = s.pop("V")
                expT = s.pop("expT")
                # row sums of exp (over k) as columns per nk + reciprocal
                pssum = ps_small.tile([128, 4], F32, name="pssum", tag="sm")
                for nk in range(4):
                    for kk in range(4):
                        nc.tensor.matmul(
                            pssum[:, nk:nk + 1],
                            expT[:, kk, ts(nk, 128)],
                            ones_c[:],
                            start=(kk == 0), stop=(kk == 3),
                        )
                recips = psc2.tile([128, 4], F32, name="recips", tag="rec")
                nc.vector.reciprocal(recips[:], pssum[:])
                # fused = softmax @ V * isd (LN folded)
                fN = pfn.tile([128, 4, D], BF16, name="fN", tag="fN")
                bn6f = psc2.tile([128, 2, 6], F32, name="bn6f", tag="bn6f")
                bnagf = psc2.tile([128, 2], F32, name="bnagf", tag="bnagf")
                for nk in range(4):
                    pfs = []
                    for dh in range(2):
                        pf = ps_main.tile([128, 512], F32, name="pf", tag="pm")
                        for kk in range(4):
                            nc.tensor.matmul(
                                pf[:],
                                expT[:, kk, ts(nk, 128)],
                                V[:, kk, dh * 512:(dh + 1) * 512],
                                start=(kk == 0), stop=(kk == 3),
                            )
                        nc.vector.bn_stats(bn6f[:, dh, :], pf[:])
                        pfs.append(pf)
                    nc.vector.bn_aggr(bnagf[:], bn6f[:])
                    s_t = psc2.tile([128, 1], F32, name="s_t", tag="s_t")
                    nc.vector.tensor_scalar(
                        s_t[:], recips[:, nk:nk + 1], float(ISD / S_WV), None,
                        op0=ALU.mult,
                    )
                    s2_t = psc2.tile([128, 1], F32, name="s2_t", tag="s2_t")
                    nc.vector.tensor_tensor(s2_t[:], s_t[:], s_t[:], op=ALU.mult)
                    vs_t = psc2.tile([128, 1], F32, name="vs_t", tag="vs_t")
                    nc.vector.scalar_tensor_tensor(
                        vs_t[:], bnagf[:, 1:2], 1.0, s2_t[:],
                        op0=ALU.mult, op1=ALU.mult,
                    )
                    i2_t = emit_rsqrt(psc2, vs_t[:], 1, "l2", EPS)
                    se_t = psc2.tile([128, 1], F32, name="se_t", tag="se_t")
                    nc.vector.tensor_tensor(se_t[:], s_t[:], i2_t[:], op=ALU.mult)
                    be_t = psc2.tile([128, 1], F32, name="be_t", tag="be_t")
                    nc.vector.scalar_tensor_tensor(
                        be_t[:], bnagf[:, 0:1], -1.0, se_t[:],
                        op0=ALU.mult, op1=ALU.mult,
                    )
                    for dh in range(2):
                        nc.scalar.activation(
                            fN[:, nk, dh * 512:(dh + 1) * 512], pfs[dh][:],
                            AF.Identity, bias=be_t[:], scale=se_t[:],
                        )
                s["fN"] = fN

            def p3(i):
                s = st[i]
                fN = s["fN"]
                fNT = pmid.tile([128, 8, C], BF16, name="fNT", tag="fNT")
                for dk in range(8):
                    ptf = ps_small.tile([128, 512], BF16, name="ptf", tag="sm")
                    for nkk in range(4):
                        nc.tensor.transpose(
                            ptf[:, ts(nkk, 128)], fN[:, nkk, ts(dk, 128)],
                            id_sb[:],
                        )
                    nc.vector.tensor_copy(fNT[:, dk, :], ptf[:])
                ph = ps_main.tile([128, C], F32, name="ph", tag="pm")
                for kc in range(8):
                    nc.tensor.matmul(
                        ph[:], wp1_sb[:, kc, :], fNT[:, kc, :],
                        start=(kc == 0), stop=(kc == 7),
                    )
                # gelu (tanh formula; Square/Tanh share the Exp table set)
                gx = pgel.tile([H, C], F32, name="gx", tag="gx")
                nc.scalar.activation(gx[:], ph[0:H, :], AF.Identity, bias=bp1_sb[:])
                g2 = pgel.tile([H, C], F32, name="g2", tag="g2")
                nc.scalar.activation(g2[:], gx[:], AF.Square)
                nc.vector.tensor_scalar(
                    g2[:], g2[:], 0.044715 * 0.7978845608028654,
                    0.7978845608028654, op0=ALU.mult, op1=ALU.add,
                )
                nc.vector.tensor_tensor(g2[:], g2[:], gx[:], op=ALU.mult)
                nc.scalar.activation(g2[:], g2[:], AF.Tanh)
                nc.vector.tensor_scalar(g2[:], g2[:], 1.0, None, op0=ALU.add)
                hT = pgel.tile([H, C], BF16, name="hT", tag="hT")
                nc.vector.scalar_tensor_tensor(
                    hT[:], g2[:], 0.5, gx[:], op0=ALU.mult, op1=ALU.mult,
                )
                s["hT"] = hT

            def p4a(i):
                s = st[i]
                hT = s.pop("hT")
                pps = ps_main.tile([1, C], F32, name="pps", tag="pm")
                nc.tensor.matmul(pps[:], wp2_sb[:], hT[:], start=True, stop=True)
                pnm = psc2.tile([1, 1], F32, name="pnm", tag="pnm")
                nc.vector.tensor_reduce(
                    pnm[:], pps[:], axis=AX.X, op=ALU.max, negate=True
                )
                pw = pgel.tile([1, C], BF16, name="pw", tag="row")
                pse = psc2.tile([1, 1], F32, name="pse", tag="pse")
                nc.scalar.activation(
                    pw[:], pps[:], AF.Exp, bias=pnm[:], accum_out=pse[:]
                )
                prc = psc2.tile([1, 1], F32, name="prc", tag="prc")
                nc.vector.reciprocal(prc[:], pse[:])
                pwn = pgel.tile([1, C], F32, name="pwn", tag="rown")
                nc.vector.tensor_scalar(
                    pwn[:], pw[:], prc[0:1, 0:1], None, op0=ALU.mult
                )
                s["pwn"] = pwn

            def p4b(i):
                s = st[i]
                pwn = s.pop("pwn")
                fN = s.pop("fN")
                ppw = ps_small.tile([128, 4], F32, name="ppw", tag="sm")
                for nk in range(4):
                    nc.tensor.transpose(
                        ppw[:, nk:nk + 1], pwn[0:1, ts(nk, 128)], idf1[:]
                    )
                pwc = pgel.tile([128, 4, 2], BF16, name="pwc", tag="pwc")
                nc.scalar.activation(pwc[:, :, 0], ppw[:, 0:4], AF.Copy)
                nc.scalar.activation(pwc[:, :, 1], ppw[:, 0:4], AF.Copy)
                for dk in range(8):
                    pp = ps_small.tile([128, 2], F32, name="pp", tag="sm")
                    for nk in range(4):
                        nc.tensor.matmul(
                            pp[:], fN[:, nk, ts(dk, 128)], pwc[:, nk, :],
                            start=(nk == 0), stop=(nk == 3),
                        )
                    nc.scalar.activation(
                        pooledT[:, dk, i:i + 1], pp[:, 0:1], AF.Identity,
                        bias=bf_sb[:, dk:dk + 1], scale=gf_sb[:, dk:dk + 1],
                    )

            for i in range(NB + 3):
                if i < NB:
                    p1_emb(i)
                if 1 <= i <= NB:
                    p2_big(i - 1)
                if 3 <= i:
                    p4a(i - 3)
                if i < NB:
                    p1_tr(i)
                if 3 <= i:
                    p4b(i - 3)
                if 1 <= i <= NB:
                    p2_fused(i - 1)
                if 2 <= i <= NB + 1:
                    p3(i - 2)

            # =================== fc head ==============================
            h1T = pmid.tile([128, 16, NB], BF16, name="h1T", tag="RT")
            h2T = pmid.tile([128, 16, NB], F32R, name="h2T", tag="fNT")
            for g in range(4):
                pg = ps_main.tile([128, 4, NB], F32, name=f"pg{g}", tag="pm")
                wts1 = []
                for kd in range(8):
                    wt = pw3.tile([128, 512], HDT, name="wt1", tag="w3")
                    nc.sync.dma_start(wt[:], WF1[g * 8 + kd])
                    wts1.append(wt)
                for j in range(4):
                    for kd in range(8):
                        nc.tensor.matmul(
                            pg[:, j, :], wts1[kd][:, ts(j, 128)],
                            pooledT[:, kd, :],
                            start=(kd == 0), stop=(kd == 7),
                        )
                for j in range(4):
                    mf = g * 4 + j
                    nc.scalar.activation(
                        h1T[:, mf, :], pg[:, j, :], AF_LRELU,
                        bias=bf1_sb[:, mf:mf + 1], alpha=0.01,
                    )

            for g in range(4):
                pg = ps_main.tile([128, 4, NB], F32, name=f"qg{g}", tag="pm")
                wts2 = []
                for kf in range(16):
                    wt = pw3.tile([128, 512], HDT, name="wt2", tag="w3")
                    nc.sync.dma_start(wt[:], WF2[g * 16 + kf])
                    wts2.append(wt)
                for j in range(4):
                    for kf in range(16):
                        nc.tensor.matmul(
                            pg[:, j, :], wts2[kf][:, ts(j, 128)], h1T[:, kf, :],
                            start=(kf == 0), stop=(kf == 15),
                        )
                for j in range(4):
                    mf = g * 4 + j
                    nc.scalar.activation(
                        h2T[:, mf, :], pg[:, j, :], AF.Identity,
                        bias=bf2_sb[:, mf:mf + 1],
                    )

            prul = ps_small.tile([NB, 2], F32, name="prul", tag="sm")
            for k in range(16):
                nc.tensor.matmul(
                    prul[:], h2T[:, k, :], wf3_sb[:, k, :],
                    start=(k == 0), stop=(k == 15),
                )
            bf3_t = psc2.tile([NB, 1], F32, name="bf3_t", tag="bf3")
            nc.gpsimd.memset(bf3_t[:], bf3_f)
            rul_sb = psc2.tile([NB, 1], F32, name="rul_sb", tag="rul")
            nc.scalar.activation(rul_sb[:], prul[:, 0:1], AF.Abs, bias=bf3_t[:])
            nc.sync.dma_start(RUL[:], rul_sb[:])

        wts.release()
        glob.release()

    nc.compile()
    return nc


def _prep_in_maps(inputs):
    f32 = np.float32
    x_enc = np.asarray(inputs["x_enc"], f32)
    W_emb = np.asarray(inputs["W_emb"], f32)
    b_emb = np.asarray(inputs["b_emb"], f32)
    g_s = np.asarray(inputs["g_s"], f32)
    b_s = np.asarray(inputs["b_s"], f32)
    basis = np.asarray(inputs["basis"], np.float64)
    Wq = np.asarray(inputs["Wq"], np.float64)
    bq = np.asarray(inputs["bq"], f32)
    Wk = np.asarray(inputs["Wk"], np.float64)
    bk = np.asarray(inputs["bk"], f32)
    Wv = np.asarray(inputs["Wv"], f32)
    bv = np.asarray(inputs["bv"], f32)
    g_f = np.asarray(inputs["g_f"], f32)
    b_f = np.asarray(inputs["b_f"], f32)
    Wp1 = np.asarray(inputs["Wp1"], f32)
    bp1 = np.asarray(inputs["bp1"], f32)
    Wp2 = np.asarray(inputs["Wp2"], f32)
    Wf1 = np.asarray(inputs["Wf1"], f32)
    bf1 = np.asarray(inputs["bf1"], f32)
    Wf2 = np.asarray(inputs["Wf2"], f32)
    bf2 = np.asarray(inputs["bf2"], f32)
    Wf3 = np.asarray(inputs["Wf3"], f32)

    assert not (np.any(bq) or np.any(bk)), "folded QK path requires bq=bk=0"

    M = (Wq @ Wk.T).astype(f32)
    Wqb = (Wq @ basis.T).astype(f32)

    wp1g = np.zeros((D, 128), f32)
    wp1g[:, :H] = g_f[:, None] * Wp1
    bp1e = (b_f @ Wp1 + bp1).reshape(H, 1).astype(f32)

    e_dt = "e4m3" if EMB_FP8 else "bf16"
    m_dt = "e4m3" if ATTN_FP8 else "bf16"
    v_dt = "e4m3" if V_FP8 else "bf16"
    h_dt = "bf16" if HEAD_DT == "bf16" else "f32r"

    common = {
        "wemb": _cast(_sbuf_layout(W_emb * S_EMB, 8), e_dt),
        "m_mat": _cast(_sbuf_layout(M * S_M, 8), m_dt),
        "wqb": _cast(_sbuf_layout(Wqb * S_WQB, 8), m_dt),
        "wv": _cast(_sbuf_layout(Wv * S_WV, 8), v_dt),
        "wp1g": _cast(_sbuf_layout(wp1g, 8), "bf16"),
        "wp2": Wp2.astype(BFNP),
        "wf1": _cast(
            Wf1.reshape(8, 128, 4, 512).transpose(2, 0, 1, 3).reshape(32, 128, 512),
            h_dt,
        ),
        "wf2": _cast(
            Wf2.reshape(16, 128, 4, 512).transpose(2, 0, 1, 3).reshape(64, 128, 512),
            h_dt,
        ),
        "wf3c": _round_f32r(
            np.repeat(_chunk_major(Wf3[:, 0], 16)[:, :, None], 2, axis=2)
        ),
        "gs_c": _chunk_major(g_s, 8),
        "bs_c": _chunk_major(b_s, 8),
        "gf_c": _chunk_major(g_f, 8),
        "bf_c": _chunk_major(b_f, 8),
        "bp1e": bp1e,
        "bf1_c": _chunk_major(bf1, 16),
        "bf2_c": _chunk_major(bf2, 16),
        "ident": np.eye(128).astype(BFNP),
    }
    bemb_nz = bool(np.any(b_emb))
    bv_nz = bool(np.any(bv))
    if bemb_nz:
        common["bemb_row"] = _round_f32r(b_emb.reshape(1, D) * S_EMB)
    if bv_nz:
        common["bv_row"] = _round_f32r(bv.reshape(1, D) * S_WV)

    in_maps = []
    for c in range(NCORES):
        m = dict(common)
        xs = x_enc[c * NB:(c + 1) * NB]
        m["x"] = _cast(
            xs.reshape(NB, 8, 128, C).transpose(0, 2, 1, 3), e_dt
        )
        in_maps.append(m)

    age_scale_f = float(np.asarray(inputs["age_scale"], f32))
    bf3_f = float(np.asarray(inputs["bf3"], f32).reshape(-1)[0])
    return in_maps, age_scale_f, bf3_f, bemb_nz, bv_nz


_NC_CACHE = {}


def build_program(inputs, sim_acts=False):
    in_maps, age_scale_f, bf3_f, bemb_nz, bv_nz = _prep_in_maps(inputs)
    key = (age_scale_f, bf3_f, bemb_nz, bv_nz, sim_acts)
    if key not in _NC_CACHE:
        _NC_CACHE[key] = _build(age_scale_f, bf3_f, bemb_nz, bv_nz, sim_acts)
    return _NC_CACHE[key], in_maps


def kernel(**inputs):
    from concourse.bass_utils import run_bass_kernel_spmd

    nc, in_maps = build_program(inputs)
    res = run_bass_kernel_spmd(nc, in_maps, core_ids=list(range(NCORES)))
    out = np.concatenate(
        [res.results[c]["rul"] for c in range(NCORES)], axis=0
    ).astype(np.float32)
    return out


# revision 4
# speedup vs baseline: 1.3917x; 1.0561x over previous
"""Trainium2 Bass kernel for nn_CLIP_69458211111620 (v2: fused pipeline).

Data-parallel over batch B=128 across 8 NeuronCores (16 batches/core).
Single fused pass per batch (no DRAM staging), software-pipelined 4 deep:
  P1(i): emb matmuls + LN + transpose -> S
  P2(i-1): RT/ab/sc/V/scoresT matmuls, exp (pre-transposed softmax), fused, LN
  P3(i-2): fNT transpose, pooling MLP hT
  P4(i-3): pool softmax + pooled columns
then fc head on SBUF-prefetched weights.

Precision: fp8 e4m3 + DoubleRow for big matmuls where the error budget
allows (sim.py ablations), bf16 elsewhere, f32r where critical.
"""
import sys

sys.path.insert(0, "/opt/trn_rl_repo")

import numpy as np
import ml_dtypes

NCORES = 8
NB = 16          # batches per core
T, C, D, DF, H = 1024, 512, 1024, 2048, 64
ISD = 1.0 / 32.0  # 1/sqrt(D)
EPS = 1e-5

# ---- precision config (validated by sim.py ablations) -----------------
# fp8 e4m3 on any attention-chain operand exceeds the 2e-2 budget (the
# softmax sharply amplifies quantization noise); all-bf16 sims at 6.6e-3.
EMB_FP8 = False   # x, wemb e4m3 (DoubleRow)
ATTN_FP8 = False  # S, M, wqb e4m3 -> RT/ab/sc DoubleRow
V_FP8 = False     # wv e4m3 (uses e4m3 S) -> V matmul DoubleRow
ABSC_DT = "bf16"  # scores matmul operand dtype ("bf16" | "f32r")
HEAD_DT = "bf16"  # fc head weight dtype ("bf16" | "f32r")

S_EMB = 64.0 if EMB_FP8 else 1.0
S_M = 16.0 if ATTN_FP8 else 1.0
S_WQB = 16.0 if ATTN_FP8 else 1.0
S_WV = 32.0 if V_FP8 else 1.0

E4NP = ml_dtypes.float8_e4m3
BFNP = ml_dtypes.bfloat16


def _round_f32r(x):
    u = np.ascontiguousarray(x, dtype=np.float32).view(np.uint32).copy()
    lsb = (u >> np.uint32(12)) & np.uint32(1)
    u += np.uint32(0x7FF) + lsb
    u &= np.uint32(0xFFFFF000)
    return u.view(np.float32)


def _chunk_major(v, nchunk):
    return np.ascontiguousarray(
        np.asarray(v, dtype=np.float32).reshape(nchunk, 128).T
    )


def _sbuf_layout(w, nk):
    """[nk*128, F] -> [128, nk, F] partition-major image."""
    w = np.asarray(w, dtype=np.float32)
    f = w.shape[1]
    return np.ascontiguousarray(
        w.reshape(nk, 128, f).transpose(1, 0, 2)
    )


def _cast(x, dt):
    if dt == "e4m3":
        return np.clip(x, -240.0, 240.0).astype(E4NP)
    if dt == "bf16":
        return np.asarray(x, np.float32).astype(BFNP)
    return _round_f32r(x)


def _build(age_scale_f, bf3_f, bemb_nz, bv_nz, sim_acts=False):
    import concourse.tile as tile
    import concourse.bass as bass
    from concourse import bacc, mybir

    F32 = mybir.dt.float32
    F32R = mybir.dt.float32r
    BF16 = mybir.dt.bfloat16
    E4M3 = mybir.dt.float8e4
    AF = mybir.ActivationFunctionType
    ALU = mybir.AluOpType
    AX = mybir.AxisListType
    PM = mybir.MatmulPerfMode
    ts = bass.ts
    AF_LRELU = AF.Relu if sim_acts else AF.Lrelu

    I32 = mybir.dt.int32
    RSQRT_MAGIC = 0x5F3759DF

    SDT = E4M3 if (ATTN_FP8 or V_FP8) else BF16
    XDT = E4M3 if EMB_FP8 else BF16
    WEDT = E4M3 if EMB_FP8 else BF16
    MDT = E4M3 if ATTN_FP8 else BF16
    WVDT = E4M3 if V_FP8 else BF16
    ABDT = F32R if ABSC_DT == "f32r" else BF16
    HDT = F32R if HEAD_DT == "f32r" else BF16

    def kch(n, fp8):
        """Chunk iteration: DoubleRow pairs if fp8 else single chunks."""
        if fp8:
            return [
                (slice(2 * p, 2 * p + 2), p == 0, p == n // 2 - 1, PM.DoubleRow)
                for p in range(n // 2)
            ]
        return [(slice(k, k + 1), k == 0, k == n - 1, None) for k in range(n)]

    nc = bacc.Bacc("TRN2", target_bir_lowering=False, debug=False)

    def inp(name, shape, dt):
        return nc.dram_tensor(name, shape, dt, kind="ExternalInput").ap()

    X = inp("x", (NB, 128, 8, C), XDT)
    WEMB = inp("wemb", (128, 8, D), WEDT)
    MQK = inp("m_mat", (128, 8, D), MDT)
    WQB = inp("wqb", (128, 8, C), MDT)
    WV = inp("wv", (128, 8, D), WVDT)
    WP1G = inp("wp1g", (128, 8, 128), BF16)
    WP2 = inp("wp2", (H, 1), BF16)
    WF1 = inp("wf1", (32, 128, 512), HDT)
    WF2 = inp("wf2", (64, 128, 512), HDT)
    WF3C = inp("wf3c", (128, DF // 128, 2), F32R)
    GS = inp("gs_c", (128, 8), F32)
    BS = inp("bs_c", (128, 8), F32)
    GF = inp("gf_c", (128, 8), F32)
    BF_ = inp("bf_c", (128, 8), F32)
    BP1E = inp("bp1e", (H, 1), F32)
    BF1 = inp("bf1_c", (128, 16), F32)
    BF2 = inp("bf2_c", (128, 16), F32)
    IDENT = inp("ident", (128, 128), BF16)
    BEMB = inp("bemb_row", (1, D), F32R) if bemb_nz else None
    BVR = inp("bv_row", (1, D), F32R) if bv_nz else None
    RUL = nc.dram_tensor("rul", (NB, 1), F32, kind="ExternalOutput").ap()

    with tile.TileContext(nc) as tc:
        # ---- long-lived tiles ----------------------------------------
        glob = tc.alloc_tile_pool(name="glob", bufs=1)
        id_sb = glob.tile([128, 128], BF16, name="id_sb")
        magic_t = glob.tile([128, 4], I32, name="magic_t")
        ages_t = glob.tile([128, 1], F32, name="ages_t")
        ones_c = glob.tile([128, 1], BF16, name="ones_c")
        idf1 = glob.tile([1, 1], F32, name="idf1")
        pooledT = glob.tile([128, 8, NB], BF16, name="pooledT")
        gs_sb = glob.tile([128, 8], F32, name="gs_sb")
        bs_sb = glob.tile([128, 8], F32, name="bs_sb")
        gf_sb = glob.tile([128, 8], F32, name="gf_sb")
        bf_sb = glob.tile([128, 8], F32, name="bf_sb")
        nc.sync.dma_start(id_sb[:], IDENT[:])
        nc.sync.dma_start(gs_sb[:], GS[:])
        nc.sync.dma_start(bs_sb[:], BS[:])
        nc.sync.dma_start(gf_sb[:], GF[:])
        nc.sync.dma_start(bf_sb[:], BF_[:])
        nc.gpsimd.memset(magic_t[:], RSQRT_MAGIC)
        nc.gpsimd.memset(ages_t[:], age_scale_f)
        nc.gpsimd.memset(ones_c[:], 1.0)
        nc.gpsimd.memset(idf1[:], 1.0)
        ones_r = None
        if bemb_nz or bv_nz:
            ones_r = glob.tile([1, 128], F32R, name="ones_r")
            nc.gpsimd.memset(ones_r[:], 1.0)
        bemb_sb = None
        if bemb_nz:
            bemb_sb = glob.tile([1, D], F32R, name="bemb_sb")
            nc.sync.dma_start(bemb_sb[:], BEMB[:])
        bv_sb = None
        if bv_nz:
            bv_sb = glob.tile([1, D], F32R, name="bv_sb")
            nc.sync.dma_start(bv_sb[:], BVR[:])

        # ---- weights --------------------------------------------------
        wts = tc.alloc_tile_pool(name="wts", bufs=1)
        wemb_sb = wts.tile([128, 8, D], WEDT, name="wemb_sb")
        m_sb = wts.tile([128, 8, D], MDT, name="m_sb")
        wqb_sb = wts.tile([128, 8, C], MDT, name="wqb_sb")
        wv_sb = wts.tile([128, 8, D], WVDT, name="wv_sb")
        wp1_sb = wts.tile([128, 8, 128], BF16, name="wp1_sb")
        wp2_sb = wts.tile([H, 1], BF16, name="wp2_sb")
        bp1_sb = wts.tile([H, 1], F32, name="bp1_sb")
        bf1_sb = wts.tile([128, 16], F32, name="bf1_sb")
        bf2_sb = wts.tile([128, 16], F32, name="bf2_sb")
        wf3_sb = wts.tile([128, 16, 2], F32R, name="wf3_sb")
        weight_dmas = [
            (m_sb, MQK), (wqb_sb, WQB), (wv_sb, WV), (wp1_sb, WP1G),
            (wp2_sb, WP2), (bp1_sb, BP1E), (bf1_sb, BF1), (bf2_sb, BF2),
            (wf3_sb, WF3C),
        ]

        def emit_rsqrt(pool, v_ap, w, tagp, eps, iters=2):
            """[128,w] -> 1/sqrt(v + eps) elementwise on DVE (Quake+Newton)."""
            ve = pool.tile([128, w], F32, name=f"{tagp}ve", tag=f"{tagp}ve")
            nc.vector.tensor_scalar(ve[:], v_ap, eps, None, op0=ALU.add)
            y = pool.tile([128, w], F32, name=f"{tagp}y0", tag=f"{tagp}y0")
            nc.vector.tensor_scalar(
                y.bitcast(I32)[:], ve.bitcast(I32)[:], 1, None,
                op0=ALU.logical_shift_right,
            )
            nc.vector.scalar_tensor_tensor(
                y.bitcast(I32)[:], y.bitcast(I32)[:], -1, magic_t[:, 0:w],
                op0=ALU.mult, op1=ALU.add,
            )
            for it in range(iters):
                a = pool.tile([128, w], F32, name=f"{tagp}a{it}", tag=f"{tagp}a{it}")
                nc.vector.tensor_tensor(a[:], y[:], y[:], op=ALU.mult)
                nc.vector.tensor_tensor(a[:], a[:], ve[:], op=ALU.mult)
                nc.vector.tensor_scalar(
                    a[:], a[:], -0.5, 1.5, op0=ALU.mult, op1=ALU.add
                )
                nc.vector.tensor_tensor(y[:], y[:], a[:], op=ALU.mult)
            return y

        # ---- pipelined main loop -------------------------------------
        with (
            tc.tile_pool(name="px", bufs=2) as px,
            tc.tile_pool(name="pw3", bufs=16) as pw3,
            tc.tile_pool(name="psen", bufs=1) as psen,
            tc.tile_pool(name="pS", bufs=2) as pS,
            tc.tile_pool(name="pmid", bufs=1) as pmid,
            tc.tile_pool(name="pfn", bufs=3) as pfn,
            tc.tile_pool(name="psc1", bufs=2) as psc1,
            tc.tile_pool(name="psc2", bufs=2) as psc2,
            tc.tile_pool(name="pgel", bufs=1) as pgel,
            tc.tile_pool(name="ps_emb", bufs=3, space="PSUM") as ps_emb,
            tc.tile_pool(name="ps_main", bufs=3, space="PSUM") as ps_main,
            tc.tile_pool(name="ps_small", bufs=2, space="PSUM") as ps_small,
        ):
            st = [dict() for _ in range(NB)]

            def p1_emb(i):
                s = st[i]
                if i == 0:
                    xb = px.tile([128, 8, C], XDT, name="xb", tag="xb")
                    nc.sync.dma_start(xb[:], X[0])
                    nc.sync.dma_start(wemb_sb[:], WEMB[:])
                    for w_t, w_d in weight_dmas:
                        nc.sync.dma_start(w_t[:], w_d[:])
                    s["xb"] = xb
                xb = s.pop("xb")
                if i + 1 < NB:
                    xb2 = px.tile([128, 8, C], XDT, name="xb2", tag="xb")
                    nc.sync.dma_start(xb2[:], X[i + 1])
                    st[i + 1]["xb"] = xb2
                sen_n = psen.tile([128, 4, D], BF16, name="sen_n", tag="sen")
                for ck in range(4):
                    bn6 = psc1.tile([128, 2, 6], F32, name="bn6", tag="st6")
                    ph2 = []
                    for dh in range(2):
                        ps_s = ps_emb.tile([128, 512], F32, name="ps_s", tag="ps_s")
                        for sl, sta, stp, pm in kch(8, EMB_FP8):
                            nc.tensor.matmul(
                                ps_s[:],
                                xb[:, sl, ts(ck, 128)],
                                wemb_sb[:, sl, dh * 512:(dh + 1) * 512],
                                start=sta,
                                stop=(stp and not bemb_nz),
                                perf_mode=pm,
                            )
                        if bemb_nz:
                            nc.tensor.matmul(
                                ps_s[:],
                                ones_r[0:1, :],
                                bemb_sb[0:1, dh * 512:(dh + 1) * 512],
                                start=False, stop=True,
                            )
                        nc.vector.bn_stats(bn6[:, dh, :], ps_s[:])
                        ph2.append(ps_s)
                    bnag = psc1.tile([128, 2], F32, name="bnag", tag="bnag")
                    nc.vector.bn_aggr(bnag[:], bn6[:])
                    i_t = emit_rsqrt(
                        psc1, bnag[:, 1:2], 1, "l1", EPS * S_EMB * S_EMB
                    )
                    negmi = psc1.tile([128, 1], F32, name="negmi", tag="negmi")
                    nc.vector.scalar_tensor_tensor(
                        negmi[:], bnag[:, 0:1], -1.0, i_t[:],
                        op0=ALU.mult, op1=ALU.mult,
                    )
                    for dh in range(2):
                        nc.scalar.activation(
                            sen_n[:, ck, dh * 512:(dh + 1) * 512],
                            ph2[dh][:], AF.Identity,
                            bias=negmi[:], scale=i_t[:],
                        )
                s["sen_n"] = sen_n

            def p1_tr(i):
                s = st[i]
                sen_n = s.pop("sen_n")
                S_t = pS.tile([128, 8, C], SDT, name="S_t", tag="S")
                for dk in range(8):
                    ps_t = ps_small.tile([128, 512], BF16, name="ps_t", tag="sm")
                    for ck in range(4):
                        nc.tensor.transpose(
                            ps_t[:, ts(ck, 128)], sen_n[:, ck, ts(dk, 128)],
                            id_sb[:],
                        )
                    nc.scalar.activation(
                        S_t[:, dk, :], ps_t[:], AF.Identity,
                        bias=bs_sb[:, dk:dk + 1], scale=gs_sb[:, dk:dk + 1],
                    )
                s["S"] = S_t

            def p2_big(i):
                s = st[i]
                S_t = s.pop("S")
                # RT = (S M)^T  [e(8), n=C]
                RT = pmid.tile([128, 8, C], MDT, name="RT", tag="RT")
                for ec in range(8):
                    ptr = ps_main.tile([128, C], F32, name="ptr", tag="pm")
                    for sl, sta, stp, pm in kch(8, ATTN_FP8):
                        nc.tensor.matmul(
                            ptr[:], m_sb[:, sl, ts(ec, 128)], S_t[:, sl, :],
                            start=sta, stop=stp, perf_mode=pm,
                        )
                    nc.scalar.activation(RT[:, ec, :], ptr[:], AF.Copy)
                # ab = S Wqb * isd/s_wqb  [n(4), m=C]
                ab = pmid.tile([128, 4, C], ABDT, name="ab", tag="ab")
                for nk in range(4):
                    pa = ps_main.tile([128, C], F32, name="pa", tag="pm")
                    for sl, sta, stp, pm in kch(8, ATTN_FP8):
                        nc.tensor.matmul(
                            pa[:], S_t[:, sl, ts(nk, 128)], wqb_sb[:, sl, :],
                            start=sta, stop=stp, perf_mode=pm,
                        )
                    nc.scalar.activation(
                        ab[:, nk, :], pa[:], AF.Copy, scale=float(ISD / S_WQB)
                    )
                # sc = R S^T * isd/s_M + age  [n(4), m=C]
                sc = pmid.tile([128, 4, C], ABDT, name="sc", tag="sc")
                for nk in range(4):
                    pa = ps_main.tile([128, C], F32, name="pa2", tag="pm")
                    for sl, sta, stp, pm in kch(8, ATTN_FP8):
                        nc.tensor.matmul(
                            pa[:], RT[:, sl, ts(nk, 128)], S_t[:, sl, :],
                            start=sta, stop=stp, perf_mode=pm,
                        )
                    nc.scalar.activation(
                        sc[:, nk, :], pa[:], AF.Identity,
                        bias=ages_t[:], scale=float(ISD / S_M),
                    )
                # V = S Wv  [m(4), D]
                V = pmid.tile([128, 4, D], BF16, name="V", tag="V")
                for mk in range(4):
                    for dh in range(2):
                        pv = ps_main.tile([128, 512], F32, name="pv", tag="pm")
                        for sl, sta, stp, pm in kch(8, V_FP8):
                            nc.tensor.matmul(
                                pv[:],
                                S_t[:, sl, ts(mk, 128)],
                                wv_sb[:, sl, dh * 512:(dh + 1) * 512],
                                start=sta,
                                stop=(stp and not bv_nz),
                                perf_mode=pm,
                            )
                        if bv_nz:
                            nc.tensor.matmul(
                                pv[:],
                                ones_r[0:1, :],
                                bv_sb[0:1, dh * 512:(dh + 1) * 512],
                                start=False, stop=True,
                            )
                        nc.vector.tensor_copy(
                            V[:, mk, dh * 512:(dh + 1) * 512], pv[:]
                        )
                s["V"] = V
                # scoresT[k,n] = sum_j sc[j,k] ab[j,n]; exp -> expT (bf16)
                expT = pmid.tile([128, 4, C], BF16, name="expT", tag="expT")
                for kk in range(4):
                    psc = ps_main.tile([128, C], F32, name="psc", tag="pm")
                    for jk in range(4):
                        nc.tensor.matmul(
                            psc[:], sc[:, jk, ts(kk, 128)], ab[:, jk, :],
                            start=(jk == 0), stop=(jk == 3),
                        )
                    nc.scalar.activation(expT[:, kk, :], psc[:], AF.Exp)
                s["expT"] = expT

            def p2_fused(i):
                s = st[i]
                V = s.pop("V")
                expT = s.pop("expT")
                # row sums of exp (over k) as columns per nk + reciprocal
                pssum = ps_small.tile([128, 4], F32, name="pssum", tag="sm")
                for nk in range(4):
                    for kk in range(4):
                        nc.tensor.matmul(
                            pssum[:, nk:nk + 1],
                            expT[:, kk, ts(nk, 128)],
                            ones_c[:],
                            start=(kk == 0), stop=(kk == 3),
                        )
                recips = psc2.tile([128, 4], F32, name="recips", tag="rec")
                nc.vector.reciprocal(recips[:], pssum[:])
                # fused = softmax @ V * isd (LN folded)
                fN = pfn.tile([128, 4, D], BF16, name="fN", tag="fN")
                bn6f = psc2.tile([128, 2, 6], F32, name="bn6f", tag="bn6f")
                bnagf = psc2.tile([128, 2], F32, name="bnagf", tag="bnagf")
                for nk in range(4):
                    pfs = []
                    for dh in range(2):
                        pf = ps_main.tile([128, 512], F32, name="pf", tag="pm")
                        for kk in range(4):
                            nc.tensor.matmul(
                                pf[:],
                                expT[:, kk, ts(nk, 128)],
                                V[:, kk, dh * 512:(dh + 1) * 512],
                                start=(kk == 0), stop=(kk == 3),
                            )
                        nc.vector.bn_stats(bn6f[:, dh, :], pf[:])
                        pfs.append(pf)
                    nc.vector.bn_aggr(bnagf[:], bn6f[:])
                    s_t = psc2.tile([128, 1], F32, name="s_t", tag="s_t")
                    nc.vector.tensor_scalar(
                        s_t[:], recips[:, nk:nk + 1], float(ISD / S_WV), None,
                        op0=ALU.mult,
                    )
                    s2_t = psc2.tile([128, 1], F32, name="s2_t", tag="s2_t")
                    nc.vector.tensor_tensor(s2_t[:], s_t[:], s_t[:], op=ALU.mult)
                    vs_t = psc2.tile([128, 1], F32, name="vs_t", tag="vs_t")
                    nc.vector.scalar_tensor_tensor(
                        vs_t[:], bnagf[:, 1:2], 1.0, s2_t[:],
                        op0=ALU.mult, op1=ALU.mult,
                    )
                    i2_t = emit_rsqrt(psc2, vs_t[:], 1, "l2", EPS)
                    se_t = psc2.tile([128, 1], F32, name="se_t", tag="se_t")
                    nc.vector.tensor_tensor(se_t[:], s_t[:], i2_t[:], op=ALU.mult)
                    be_t = psc2.tile([128, 1], F32, name="be_t", tag="be_t")
                    nc.vector.scalar_tensor_tensor(
                        be_t[:], bnagf[:, 0:1], -1.0, se_t[:],
                        op0=ALU.mult, op1=ALU.mult,
                    )
                    for dh in range(2):
                        nc.scalar.activation(
                            fN[:, nk, dh * 512:(dh + 1) * 512], pfs[dh][:],
                            AF.Identity, bias=be_t[:], scale=se_t[:],
                        )
                s["fN"] = fN

            def p3(i):
                s = st[i]
                fN = s["fN"]
                fNT = pmid.tile([128, 8, C], BF16, name="fNT", tag="fNT")
                for dk in range(8):
                    ptf = ps_small.tile([128, 512], BF16, name="ptf", tag="sm")
                    for nkk in range(4):
                        nc.tensor.transpose(
                            ptf[:, ts(nkk, 128)], fN[:, nkk, ts(dk, 128)],
                            id_sb[:],
                        )
                    nc.vector.tensor_copy(fNT[:, dk, :], ptf[:])
                ph = ps_main.tile([128, C], F32, name="ph", tag="pm")
                for kc in range(8):
                    nc.tensor.matmul(
                        ph[:], wp1_sb[:, kc, :], fNT[:, kc, :],
                        start=(kc == 0), stop=(kc == 7),
                    )
                # gelu (tanh formula; Square/Tanh share the Exp table set)
                gx = pgel.tile([H, C], F32, name="gx", tag="gx")
                nc.scalar.activation(gx[:], ph[0:H, :], AF.Identity, bias=bp1_sb[:])
                g2 = pgel.tile([H, C], F32, name="g2", tag="g2")
                nc.scalar.activation(g2[:], gx[:], AF.Square)
                nc.vector.tensor_scalar(
                    g2[:], g2[:], 0.044715 * 0.7978845608028654,
                    0.7978845608028654, op0=ALU.mult, op1=ALU.add,
                )
                nc.vector.tensor_tensor(g2[:], g2[:], gx[:], op=ALU.mult)
                nc.scalar.activation(g2[:], g2[:], AF.Tanh)
                nc.vector.tensor_scalar(g2[:], g2[:], 1.0, None, op0=ALU.add)
                hT = pgel.tile([H, C], BF16, name="hT", tag="hT")
                nc.vector.scalar_tensor_tensor(
                    hT[:], g2[:], 0.5, gx[:], op0=ALU.mult, op1=ALU.mult,
                )
                s["hT"] = hT

            def p4a(i):
                s = st[i]
                hT = s.pop("hT")
                pps = ps_main.tile([1, C], F32, name="pps", tag="pm")
                nc.tensor.matmul(pps[:], wp2_sb[:], hT[:], start=True, stop=True)
                pnm = psc2.tile([1, 1], F32, name="pnm", tag="pnm")
                nc.vector.tensor_reduce(
                    pnm[:], pps[:], axis=AX.X, op=ALU.max, negate=True
                )
                pw = pgel.tile([1, C], BF16, name="pw", tag="row")
                pse = psc2.tile([1, 1], F32, name="pse", tag="pse")
                nc.scalar.activation(
                    pw[:], pps[:], AF.Exp, bias=pnm[:], accum_out=pse[:]
                )
                prc = psc2.tile([1, 1], F32, name="prc", tag="prc")
                nc.vector.reciprocal(prc[:], pse[:])
                pwn = pgel.tile([1, C], F32, name="pwn", tag="rown")
                nc.vector.tensor_scalar(
                    pwn[:], pw[:], prc[0:1, 0:1], None, op0=ALU.mult
                )
                s["pwn"] = pwn

            def p4b(i):
                s = st[i]
                pwn = s.pop("pwn")
                fN = s.pop("fN")
                ppw = ps_small.tile([128, 4], F32, name="ppw", tag="sm")
                for nk in range(4):
                    nc.tensor.transpose(
                        ppw[:, nk:nk + 1], pwn[0:1, ts(nk, 128)], idf1[:]
                    )
                pwc = pgel.tile([128, 4, 2], BF16, name="pwc", tag="pwc")
                nc.scalar.activation(pwc[:, :, 0], ppw[:, 0:4], AF.Copy)
                nc.scalar.activation(pwc[:, :, 1], ppw[:, 0:4], AF.Copy)
                for dk in range(8):
                    pp = ps_small.tile([128, 2], F32, name="pp", tag="sm")
                    for nk in range(4):
                        nc.tensor.matmul(
                            pp[:], fN[:, nk, ts(dk, 128)], pwc[:, nk, :],
                            start=(nk == 0), stop=(nk == 3),
                        )
                    nc.scalar.activation(
                        pooledT[:, dk, i:i + 1], pp[:, 0:1], AF.Identity,
                        bias=bf_sb[:, dk:dk + 1], scale=gf_sb[:, dk:dk + 1],
                    )

            for i in range(NB + 3):
                if i < NB:
                    p1_emb(i)
                if 1 <= i <= NB:
                    p2_big(i - 1)
                if 3 <= i:
                    p4a(i - 3)
                if 1 <= i <= NB:
                    p2_fused(i - 1)
                if 3 <= i:
                    p4b(i - 3)
                if i < NB:
                    p1_tr(i)
                if 2 <= i <= NB + 1:
                    p3(i - 2)

            # =================== fc head ==============================
            h1T = pmid.tile([128, 16, NB], BF16, name="h1T", tag="RT")
            h2T = pmid.tile([128, 16, NB], F32R, name="h2T", tag="fNT")
            for g in range(4):
                pg = ps_main.tile([128, 4, NB], F32, name=f"pg{g}", tag="pm")
                wts1 = []
                for kd in range(8):
                    wt = pw3.tile([128, 512], HDT, name="wt1", tag="w3")
                    nc.sync.dma_start(wt[:], WF1[g * 8 + kd])
                    wts1.append(wt)
                for j in range(4):
                    for kd in range(8):
                        nc.tensor.matmul(
                            pg[:, j, :], wts1[kd][:, ts(j, 128)],
                            pooledT[:, kd, :],
                            start=(kd == 0), stop=(kd == 7),
                        )
                for j in range(4):
                    mf = g * 4 + j
                    nc.scalar.activation(
                        h1T[:, mf, :], pg[:, j, :], AF_LRELU,
                        bias=bf1_sb[:, mf:mf + 1], alpha=0.01,
                    )

            for g in range(4):
                pg = ps_main.tile([128, 4, NB], F32, name=f"qg{g}", tag="pm")
                wts2 = []
                for kf in range(16):
                    wt = pw3.tile([128, 512], HDT, name="wt2", tag="w3")
                    nc.sync.dma_start(wt[:], WF2[g * 16 + kf])
                    wts2.append(wt)
                for j in range(4):
                    for kf in range(16):
                        nc.tensor.matmul(
                            pg[:, j, :], wts2[kf][:, ts(j, 128)], h1T[:, kf, :],
                            start=(kf == 0), stop=(kf == 15),
                        )
                for j in range(4):
                    mf = g * 4 + j
                    nc.scalar.activation(
                        h2T[:, mf, :], pg[:, j, :], AF.Identity,
                        bias=bf2_sb[:, mf:mf + 1],
                    )

            prul = ps_small.tile([NB, 2], F32, name="prul", tag="sm")
            for k in range(16):
                nc.tensor.matmul(
                    prul[:], h2T[:, k, :], wf3_sb[:, k, :],
                    start=(k == 0), stop=(k == 15),
                )
            bf3_t = psc2.tile([NB, 1], F32, name="bf3_t", tag="bf3")
            nc.gpsimd.memset(bf3_t[:], bf3_f)
            rul_sb = psc2.tile([NB, 1], F32, name="rul_sb", tag="rul")
            nc.scalar.activation(rul_sb[:], prul[:, 0:1], AF.Abs, bias=bf3_t[:])
            nc.sync.dma_start(RUL[:], rul_sb[:])

        wts.release()
        glob.release()

    nc.compile()
    return nc


def _prep_in_maps(inputs):
    f32 = np.float32
    x_enc = np.asarray(inputs["x_enc"], f32)
    W_emb = np.asarray(inputs["W_emb"], f32)
    b_emb = np.asarray(inputs["b_emb"], f32)
    g_s = np.asarray(inputs["g_s"], f32)
    b_s = np.asarray(inputs["b_s"], f32)
    basis = np.asarray(inputs["basis"], np.float64)
    Wq = np.asarray(inputs["Wq"], np.float64)
    bq = np.asarray(inputs["bq"], f32)
    Wk = np.asarray(inputs["Wk"], np.float64)
    bk = np.asarray(inputs["bk"], f32)
    Wv = np.asarray(inputs["Wv"], f32)
    bv = np.asarray(inputs["bv"], f32)
    g_f = np.asarray(inputs["g_f"], f32)
    b_f = np.asarray(inputs["b_f"], f32)
    Wp1 = np.asarray(inputs["Wp1"], f32)
    bp1 = np.asarray(inputs["bp1"], f32)
    Wp2 = np.asarray(inputs["Wp2"], f32)
    Wf1 = np.asarray(inputs["Wf1"], f32)
    bf1 = np.asarray(inputs["bf1"], f32)
    Wf2 = np.asarray(inputs["Wf2"], f32)
    bf2 = np.asarray(inputs["bf2"], f32)
    Wf3 = np.asarray(inputs["Wf3"], f32)

    assert not (np.any(bq) or np.any(bk)), "folded QK path requires bq=bk=0"

    M = (Wq @ Wk.T).astype(f32)
    Wqb = (Wq @ basis.T).astype(f32)

    wp1g = np.zeros((D, 128), f32)
    wp1g[:, :H] = g_f[:, None] * Wp1
    bp1e = (b_f @ Wp1 + bp1).reshape(H, 1).astype(f32)

    e_dt = "e4m3" if EMB_FP8 else "bf16"
    m_dt = "e4m3" if ATTN_FP8 else "bf16"
    v_dt = "e4m3" if V_FP8 else "bf16"
    h_dt = "bf16" if HEAD_DT == "bf16" else "f32r"

    common = {
        "wemb": _cast(_sbuf_layout(W_emb * S_EMB, 8), e_dt),
        "m_mat": _cast(_sbuf_layout(M * S_M, 8), m_dt),
        "wqb": _cast(_sbuf_layout(Wqb * S_WQB, 8), m_dt),
        "wv": _cast(_sbuf_layout(Wv * S_WV, 8), v_dt),
        "wp1g": _cast(_sbuf_layout(wp1g, 8), "bf16"),
        "wp2": Wp2.astype(BFNP),
        "wf1": _cast(
            Wf1.reshape(8, 128, 4, 512).transpose(2, 0, 1, 3).reshape(32, 128, 512),
            h_dt,
        ),
        "wf2": _cast(
            Wf2.reshape(16, 128, 4, 512).transpose(2, 0, 1, 3).reshape(64, 128, 512),
            h_dt,
        ),
        "wf3c": _round_f32r(
            np.repeat(_chunk_major(Wf3[:, 0], 16)[:, :, None], 2, axis=2)
        ),
        "gs_c": _chunk_major(g_s, 8),
        "bs_c": _chunk_major(b_s, 8),
        "gf_c": _chunk_major(g_f, 8),
        "bf_c": _chunk_major(b_f, 8),
        "bp1e": bp1e,
        "bf1_c": _chunk_major(bf1, 16),
        "bf2_c": _chunk_major(bf2, 16),
        "ident": np.eye(128).astype(BFNP),
    }
    bemb_nz = bool(np.any(b_emb))
    bv_nz = bool(np.any(bv))
    if bemb_nz:
        common["bemb_row"] = _round_f32r(b_emb.reshape(1, D) * S_EMB)
    if bv_nz:
        common["bv_row"] = _round_f32r(bv.reshape(1, D) * S_WV)

    in_maps = []
    for c in range(NCORES):
        m = dict(common)
        xs = x_enc[c * NB:(c + 1) * NB]
        m["x"] = _cast(
            xs.reshape(NB, 8, 128, C).transpose(0, 2, 1, 3), e_dt
        )
        in_maps.append(m)

    age_scale_f = float(np.asarray(inputs["age_scale"], f32))
    bf3_f = float(np.asarray(inputs["bf3"], f32).reshape(-1)[0])
    return in_maps, age_scale_f, bf3_f, bemb_nz, bv_nz


_NC_CACHE = {}


def build_program(inputs, sim_acts=False):
    in_maps, age_scale_f, bf3_f, bemb_nz, bv_nz = _prep_in_maps(inputs)
    key = (age_scale_f, bf3_f, bemb_nz, bv_nz, sim_acts)
    if key not in _NC_CACHE:
        _NC_CACHE[key] = _build(age_scale_f, bf3_f, bemb_nz, bv_nz, sim_acts)
    return _NC_CACHE[key], in_maps


def kernel(**inputs):
    from concourse.bass_utils import run_bass_kernel_spmd

    nc, in_maps = build_program(inputs)
    res = run_bass_kernel_spmd(nc, in_maps, core_ids=list(range(NCORES)))
    out = np.concatenate(
        [res.results[c]["rul"] for c in range(NCORES)], axis=0
    ).astype(np.float32)
    return out
